# revision 1
# baseline (speedup 1.0000x reference)
"""AppUsageFEDformer Trainium2 kernel — 8-core data-parallel Bass implementation.

Strategy: pure data parallelism over batch (64 -> 8 per NeuronCore).  Each
core runs the full model on its batch shard:
  embedding gather -> 2x encoder layer (Q-proj, Fourier block via DFT
  matmuls, mode mix, iDFT, out-proj, series-decomp as banded matmul, FFN)
  -> final my_layernorm -> vocab projection.

Layouts per core (b = local batch 0..7, l = seq 0..511, tokens t = b*512+l):
  x_S  (spine, f32):  SBUF [128(l%128), (b, c=l//128, d)]  "S layout"
  x_T  (bf16):        SBUF [128(d%128), (dc=d//128, t)]    "T layout",
                      built via cast-DMA to DRAM + xbar DMA-transpose.
All matmuls run in bf16 (f32 PSUM accumulate); the f32 spine only ever
receives f32 adds/subtracts, so precision is dominated by the bf16
rounding of branch outputs, each of which is small relative to x.
All weights are pre-arranged on the host into stationary/moving layouts so
device code only does contiguous DMA loads.
"""

import os

import numpy as np
import ml_dtypes

import concourse.bass as bass
import concourse.tile as tile
from concourse import mybir
from concourse.bass_utils import run_bass_kernel_spmd


# ------------------------------------------------------------------
# BIR wait-legalizer (inlined): the axon-path walrus rejects
# instructions carrying more than one sem wait ("Too many sync wait
# commands").  Hoist excess waits onto injected same-engine Drains.

import json

_CAP = 1  # max waits left on any real instruction


def _mk_nop(engine, name, waits):
    return {
        "debug": 0,
        "engine": engine,
        "ins": [],
        "name": name,
        "opcode": "Drain",
        "outs": [],
        "sync_info": {"on_update": [], "on_wait": waits},
    }


def legalize_bir_waits(bir_json: bytes) -> bytes:
    d = json.loads(bir_json)
    ctr = [0]
    changed = [0]
    for fn in d.get("functions", []):
        for bb in fn.get("blocks", []):
            insts = bb.get("instructions")
            if not insts:
                continue
            out = []
            for inst in insts:
                si = inst.get("sync_info")
                waits = (si or {}).get("on_wait") or []
                if len(waits) > _CAP:
                    keep = waits[:_CAP]
                    excess = waits[_CAP:]
                    for w in excess:
                        ctr[0] += 1
                        out.append(_mk_nop(inst["engine"], f"I-nopw{ctr[0]}", [w]))
                    si["on_wait"] = keep
                    changed[0] += 1
                out.append(inst)
            bb["instructions"] = out
    return json.dumps(d).encode()


def install():
    """Patch concourse.bass2jax.compile_bir_kernel to legalize first."""
    import concourse.bass2jax as b2j
    if getattr(b2j, "_legalize_installed", False):
        return
    orig = b2j.compile_bir_kernel

    def wrapped(bir_json, tmpdir, neff_name="file.neff"):
        return orig(legalize_bir_waits(bir_json), tmpdir, neff_name)

    b2j.compile_bir_kernel = wrapped
    b2j._legalize_installed = True


F32 = mybir.dt.float32
BF16 = mybir.dt.bfloat16
I32 = mybir.dt.int32
I16 = mybir.dt.int16
AF = mybir.ActivationFunctionType
ALU = mybir.AluOpType

B, L, D, H, DFF, MODES, ELAYERS = 64, 512, 512, 8, 2048, 32, 2
VOCAB, NUM_APP, KERNEL = 10000, 10000, 25
E = D // H  # 64
NCORES = 8
BL = B // NCORES  # 8 local batch
NT = BL * L       # 4096 local tokens
bf16 = ml_dtypes.bfloat16


# ---------------------------------------------------------------- host prep

def _movavg_matrix():
    """M[l_src, l_out]: weight of x[l_src] in moving_avg[l_out], including
    edge replication (pad (K-1)//2 each side with edge values)."""
    M = np.zeros((L, L), np.float64)
    pad = (KERNEL - 1) // 2
    for lo in range(L):
        for j in range(lo - pad, lo + pad + 1):
            M[min(max(j, 0), L - 1), lo] += 1.0 / KERNEL
    return M.astype(np.float32)


def _dft_c4():
    """C4 [128, (4 c, 64)]: stationary for DFT.  col j<32: cos(2pi*l*j/L);
    j>=32: -sin(2pi*l*(j-32)/L) with l = c*128+p."""
    out = np.zeros((128, 4, 64), np.float32)
    for c in range(4):
        lv = c * 128 + np.arange(128)
        for m in range(MODES):
            ang = 2.0 * np.pi * lv * m / L
            out[:, c, m] = np.cos(ang)
            out[:, c, 32 + m] = -np.sin(ang)
    return out.astype(bf16)


def _idft_d():
    """Dstack [64, 512]: rows m<32: sc(m)*cos(2pi*m*l'/L); rows 32+m:
    -sc(m)*sin(...), sc = (2-delta_m0)/L."""
    out = np.zeros((64, L), np.float32)
    lp = np.arange(L)
    for m in range(MODES):
        sc = (1.0 if m == 0 else 2.0) / L
        ang = 2.0 * np.pi * m * lp / L
        out[m] = sc * np.cos(ang)
        out[32 + m] = -sc * np.sin(ang)
    return out.astype(bf16)


def prep_weights(inp):
    """Pre-arrange all weights into SBUF-shaped host arrays."""
    w = {}
    w["embt"] = (np.asarray(inp["app_emb_w"], np.float32)
                 + np.asarray(inp["time_b"], np.float32)[None, :]
                 ).astype(bf16)
    w["tw_rep"] = np.broadcast_to(
        np.asarray(inp["time_w"], np.float32), (128, D)).copy()
    w["c4"] = _dft_c4()
    w["dstack"] = _idft_d()

    M = _movavg_matrix()
    adiag = np.zeros((128, 4, 128), np.float32)
    for c in range(4):
        adiag[:, c, :] = M[c * 128:(c + 1) * 128, c * 128:(c + 1) * 128]
    w["adiag"] = adiag.astype(bf16)
    # band tiles zero-padded to 32/64-aligned partition bases (matmul
    # requires base_partition in {0, 32, 64} matching on both operands)
    aup = np.zeros((32, 3, 128), np.float32)    # tile (co+1, co): rows 0:12
    alo = np.zeros((128, 3, 128), np.float32)   # tile (co-1, co): rows 116:128
    for co in range(3):
        aup[0:12, co, :] = M[(co + 1) * 128:(co + 1) * 128 + 12,
                             co * 128:(co + 1) * 128]
    for co in range(1, 4):
        alo[116:128, co - 1, :] = M[co * 128 - 12:co * 128,
                                    co * 128:(co + 1) * 128]
    w["aup"] = aup.astype(bf16)
    w["alo"] = alo.astype(bf16)

    Wq = np.asarray(inp["Wq"], np.float32)
    Wo = np.asarray(inp["Wo"], np.float32)
    wr = np.asarray(inp["four_wr"], np.float32)
    wi = np.asarray(inp["four_wi"], np.float32)
    c1 = np.asarray(inp["conv1_w"], np.float32)
    c2 = np.asarray(inp["conv2_w"], np.float32)
    bq = np.asarray(inp["bq"], np.float32)
    bo = np.asarray(inp["bo"], np.float32)

    for l in range(ELAYERS):
        wqt = np.zeros((128, 4, D), np.float32)
        for k in range(4):
            wqt[:, k, :] = Wq[l].T[k * 128:(k + 1) * 128, :]
        w[f"wqt{l}"] = wqt.astype(bf16)
        # bq is folded into the DFT output bias: xs += sum_l C[l,m] * bq
        # (only the m=0 cos row survives: 512*bq).  bo cancels exactly in
        # series_decomp (moving_avg of a constant is the constant), so the
        # entire +bo add is dropped.
        xs_bias = np.zeros((64, 2, H, 32), np.float32)   # [m_ri,(q,h,i32)]
        xs_bias[0, :, :, :] = (512.0 * bq[l].reshape(H, 2, 32)
                               ).transpose(1, 0, 2)
        w[f"xsb{l}"] = xs_bias

        wc = np.zeros((128, H, MODES, 128), np.float32)
        for h in range(H):
            for m in range(MODES):
                wc[0:64, h, m, 0:64] = wr[l, h, :, :, m]     # i x o
                wc[0:64, h, m, 64:128] = wi[l, h, :, :, m]
                wc[64:128, h, m, 0:64] = -wi[l, h, :, :, m]
                wc[64:128, h, m, 64:128] = wr[l, h, :, :, m]
        w[f"wc{l}"] = wc.astype(bf16)

        wot = np.zeros((128, 4, D), np.float32)
        for jc in range(4):
            wot[:, jc, :] = Wo[l].T[jc * 128:(jc + 1) * 128, :]
        w[f"wot{l}"] = wot.astype(bf16)

        c1s = np.zeros((128, 4, 16, 128), np.float32)
        for dc in range(4):
            for ft in range(16):
                c1s[:, dc, ft, :] = c1[l][ft * 128:(ft + 1) * 128,
                                          dc * 128:(dc + 1) * 128].T
        w[f"c1{l}"] = c1s.astype(bf16)

        c2t = np.zeros((128, 16, D), np.float32)
        for fc in range(16):
            c2t[:, fc, :] = c2[l].T[fc * 128:(fc + 1) * 128, :]
        w[f"c2{l}"] = c2t.astype(bf16)

    w["normw"] = np.broadcast_to(
        np.asarray(inp["norm_w"], np.float32), (128, D)).copy()
    pw = np.asarray(inp["proj_w"], np.float32)       # [10000, 536]
    pwt = np.zeros((128, 4, NUM_APP), np.float32)
    for ck in range(4):
        pwt[:, ck, :] = pw.T[ck * 128:(ck + 1) * 128, :]
    w["pwt"] = pwt.astype(bf16)
    w["pwt4"] = pw.T[512:536, :].astype(bf16)        # [24, 10000]
    w["pb"] = np.broadcast_to(
        np.asarray(inp["proj_b"], np.float32), (8, NUM_APP)).copy()
    w["ones"] = np.ones((128, 1), np.float32).astype(bf16)
    w["ident8"] = np.eye(8, dtype=np.float32)
    return w


# ---------------------------------------------------------------- builder

def _decomp(nc, p_ps, p_xbf, x_S, adiag_t, aup_t, alo_t, post_b=None):
    """x_S <- x_S - moving_avg(x_S) via banded bf16 matmul over l.
    A per-b bf16 copy of x_S serves as the matmul moving operand.
    post_b(b) emits per-batch epilogue ops right after batch b's update
    (used to pipeline the final layernorm stats into the last decomp)."""
    for b in range(BL):
        x_bf = p_xbf.tile([128, 4, D], BF16, tag="xbf")
        for c in range(4):
            nc.vector.tensor_copy(x_bf[:, c, :], x_S[:, b, c, :])
        for co in range(4):
            mms = [(adiag_t[:, co, :], x_bf[:, co, :])]
            if co < 3:
                mms.append((aup_t[:, co, :], x_bf[0:32, co + 1, :]))
            if co > 0:
                mms.append((alo_t[64:128, co - 1, :],
                            x_bf[64:128, co - 1, :]))
            ps_a = p_ps.tile([128, D], F32, tag="ps")
            for i, (lhsT, rhs) in enumerate(mms):
                nc.tensor.matmul(ps_a[:], lhsT, rhs,
                                 start=(i == 0), stop=(i == len(mms) - 1))
            nc.vector.tensor_sub(x_S[:, b, co, :], x_S[:, b, co, :], ps_a[:])
        if post_b is not None:
            post_b(b, x_bf)


def build_nc(num_devices=NCORES, debug=False):
    nc = bass.Bass("TRN2", target_bir_lowering=False, debug=False,
                   num_devices=num_devices)
    P = {}

    def param(name, shape, dtype):
        P[name] = nc.declare_dram_parameter(name, list(shape), dtype,
                                            isOutput=False)

    param("x_app", [BL, L], I32)
    param("x_time", [BL, L], F32)
    param("tv_last", [BL, 24], F32)
    param("embt", [VOCAB, D], BF16)
    param("tw_rep", [128, D], F32)
    param("c4", [128, 4, 64], BF16)
    param("dstack", [64, L], BF16)
    param("adiag", [128, 4, 128], BF16)
    param("aup", [32, 3, 128], BF16)
    param("alo", [128, 3, 128], BF16)
    for l in range(ELAYERS):
        param(f"wqt{l}", [128, 4, D], BF16)
        param(f"xsb{l}", [64, 2, H, 32], F32)
        param(f"wc{l}", [128, H, MODES, 128], BF16)
        param(f"wot{l}", [128, 4, D], BF16)
        param(f"c1{l}", [128, 4, 16, 128], BF16)
        param(f"c2{l}", [128, 16, D], BF16)
    param("normw", [128, D], F32)
    param("pwt", [128, 4, NUM_APP], BF16)
    param("pwt4", [24, NUM_APP], BF16)
    param("pb", [8, NUM_APP], F32)
    param("ones", [128, 1], BF16)
    param("ident8", [8, 8], F32)

    OUT = nc.declare_dram_parameter("out", [BL, NUM_APP], F32, isOutput=True)
    xrow = nc.dram_tensor("xrow", [NT, D], BF16)   # transpose bounce

    dbg = {}

    def dbg_dump(name, ap_or_tile, shape, dtype=F32):
        if debug:
            dbg[name] = nc.declare_dram_parameter(
                "dbg_" + name, list(shape), dtype, isOutput=True)
            nc.sync.dma_start(dbg[name].ap(), ap_or_tile)

    with tile.TileContext(nc) as tc:
        with tc.tile_pool(name="spine", bufs=1) as p_spine, \
             tc.tile_pool(name="bigT", bufs=1) as p_bigT, \
             tc.tile_pool(name="xbf", bufs=1) as p_xbf, \
             tc.tile_pool(name="qsb", bufs=1) as p_qsb, \
             tc.tile_pool(name="four", bufs=1) as p_four, \
             tc.tile_pool(name="wcb", bufs=2) as p_wcb, \
             tc.tile_pool(name="wts", bufs=1) as p_wts, \
             tc.tile_pool(name="wly", bufs=1) as p_wly, \
             tc.tile_pool(name="sml", bufs=1) as p_sml, \
             tc.tile_pool(name="pwp", bufs=2) as p_pw, \
             tc.tile_pool(name="pws", bufs=1) as p_pws, \
             tc.tile_pool(name="ps", bufs=6, space="PSUM") as p_ps, \
             tc.tile_pool(name="ps2", bufs=2, space="PSUM") as p_ps2:

            # ---------------- constants / shared weights ----------------
            c4_t = p_wts.tile([128, 4, 64], BF16, tag="c4")
            nc.sync.dma_start(c4_t[:], P["c4"][:])
            dst_t = p_wts.tile([64, L], BF16, tag="dstack")
            nc.sync.dma_start(dst_t[:], P["dstack"][:])
            adiag_t = p_wts.tile([128, 4, 128], BF16, tag="adiag")
            nc.sync.dma_start(adiag_t[:], P["adiag"][:])
            aup_t = p_wts.tile([32, 3, 128], BF16, tag="aup")
            nc.sync.dma_start(aup_t[:], P["aup"][:])
            alo_t = p_wts.tile([128, 3, 128], BF16, tag="alo")
            nc.sync.dma_start(alo_t[:], P["alo"][:])
            tw_t = p_wts.tile([128, D], F32, tag="twrep")
            nc.sync.dma_start(tw_t[:], P["tw_rep"][:])
            ones_t = p_wts.tile([128, 1], BF16, tag="ones")
            nc.sync.dma_start(ones_t[:], P["ones"][:])
            id8_t = p_wts.tile([8, 8], F32, tag="id8")
            nc.sync.dma_start(id8_t[:], P["ident8"][:])
            normw_t = p_wts.tile([128, D], F32, tag="normw")
            nc.sync.dma_start(normw_t[:], P["normw"][:])

            # ---------------- embedding ----------------
            idx_sb = p_sml.tile([128, 32], I32, tag="idxsb")
            # idx_sb[p, b*4+c] = x_app[b, c*128+p]  (= token t's app id)
            nc.sync.dma_start(
                idx_sb[:],
                P["x_app"].ap().rearrange("b (c p) -> p (b c)", p=128))

            x_S = p_spine.tile([128, BL, 4, D], F32, tag="spine")
            for b in range(BL):
                for c in range(4):
                    j = b * 4 + c
                    nc.gpsimd.indirect_dma_start(
                        out=x_S[:, b, c, :], out_offset=None,
                        in_=P["embt"][:],
                        in_offset=bass.IndirectOffsetOnAxis(
                            ap=idx_sb[:, j:j + 1], axis=0))

            xt_t = p_sml.tile([128, 32], F32, tag="xt")
            nc.sync.dma_start(
                xt_t[:],
                P["x_time"].ap().rearrange("b (c p) -> p (b c)", p=128))
            for b in range(BL):
                for c in range(4):
                    j = b * 4 + c
                    nc.vector.scalar_tensor_tensor(
                        out=x_S[:, b, c, :], in0=tw_t[:],
                        scalar=xt_t[:, j:j + 1], in1=x_S[:, b, c, :],
                        op0=ALU.mult, op1=ALU.add)

            dbg_dump("x0", x_S[:], [128, BL, 4, D])

            # ---------------- layers ----------------
            for l in range(ELAYERS):
                wqt_t = p_wly.tile([128, 4, D], BF16, tag="wqt")
                nc.sync.dma_start(wqt_t[:], P[f"wqt{l}"][:])
                xsb_t = p_wly.tile([64, 2, H, 32], F32, tag="xsb")
                nc.sync.dma_start(xsb_t[:], P[f"xsb{l}"][:])
                wot_t = p_wly.tile([128, 4, D], BF16, tag="wot")
                nc.sync.dma_start(wot_t[:], P[f"wot{l}"][:])

                # ---- x_T via cast-DMA to DRAM + xbar transpose ----
                for b in range(BL):
                    for c in range(4):
                        t0 = (b * 4 + c) * 128
                        nc.gpsimd.dma_start(xrow[t0:t0 + 128, :],
                                            x_S[:, b, c, :])
                x_T = p_bigT.tile([128, 4, NT], BF16, tag="bigT")
                for k in range(4):
                    nc.sync.dma_start_transpose(
                        x_T[:, k, :], xrow[:, k * 128:(k + 1) * 128])

                # ---- Q-proj + DFT (per b) ----
                xs_sb = p_four.tile([64, BL, 2, H, 32], BF16, tag="xs")
                for b in range(BL):
                    q_sb = p_qsb.tile([128, 4, D], BF16, tag="qsb")
                    for c in range(4):
                        t0 = (b * 4 + c) * 128
                        ps_q = p_ps.tile([128, D], F32, tag="ps")
                        for k in range(4):
                            nc.tensor.matmul(
                                ps_q[:], x_T[:, k, t0:t0 + 128],
                                wqt_t[:, k, :],
                                start=(k == 0), stop=(k == 3))
                        nc.scalar.copy(q_sb[:, c, :], ps_q[:])
                    ps_xs = p_ps2.tile([64, D], F32, tag="psxs")
                    for c in range(4):
                        nc.tensor.matmul(ps_xs[:], c4_t[:, c, :],
                                         q_sb[:, c, :],
                                         start=(c == 0), stop=(c == 3))
                    # psum free order (h, q, i32) -> xs_sb[:, b, q, h, i32]
                    nc.vector.tensor_add(
                        xs_sb[:, b, :, :, :].rearrange("p q h i -> p h q i"),
                        ps_xs[:].rearrange("p (h q i) -> p h q i", h=8, q=2),
                        xsb_t[:].rearrange("p q h i -> p h q i"))

                # ---- shuffle1: xs -> xsT [(ri,i), (b, h, m)] ----
                xsT = p_qsb.tile([128, BL, H, 32], BF16, tag="qsb")
                for b in range(BL):
                    for ri in range(2):
                        for q in range(2):
                            r0 = ri * 64 + q * 32
                            nc.vector.transpose(
                                xsT[r0:r0 + 32, b, :, :]
                                .rearrange("p h m -> p (h m)"),
                                xs_sb[ri * 32:ri * 32 + 32, b, q, :, :]
                                .rearrange("p h i -> p (h i)"))

                # ---- mode mix ----
                # psum tile (mh, hp): [128, 512]; rows 32g+b (g = h//2);
                # cols mi*128 + (ri*64 + o).  modes_sb rows sparse 32g+b.
                modes_sb = p_four.tile([128, 2, 2, 64, MODES], BF16,
                                       tag="modes")
                nc.vector.memset(modes_sb[:], 0.0)
                for mh in range(8):
                    m0 = mh * 4
                    wc_t = p_wcb.tile([128, H, 4, 128], BF16, tag="wcb")
                    nc.sync.dma_start(
                        wc_t[:], P[f"wc{l}"][:, :, m0:m0 + 4, :])
                    for hp in range(2):
                        ps_mm = p_ps.tile([128, D], F32, tag="ps")
                        for g in range(4):
                            h = g * 2 + hp
                            for mi in range(4):
                                nc.tensor.matmul(
                                    ps_mm[32 * g:32 * g + 8,
                                          mi * 128:(mi + 1) * 128],
                                    xsT[:, :, h, m0 + mi],
                                    wc_t[:, h, mi, :],
                                    start=True, stop=True,
                                    tile_position=(0, 32 * g))
                        for g in range(4):
                            nc.scalar.copy(
                                modes_sb[32 * g:32 * g + 8, hp, :, :,
                                         m0:m0 + 4]
                                .rearrange("p r o m -> p m r o"),
                                ps_mm[32 * g:32 * g + 8, :]
                                .rearrange("p (m r o) -> p m r o",
                                           m=4, r=2))

                # ---- shuffle2 / iDFT / Wo, pipelined in g-pair halves:
                # the DVE stream-transposes of half gp overlap the PE
                # iDFT+Wo matmuls of half gp-1 ----
                modes_T = p_four.tile([64, 2, 64, 32], BF16, tag="modesT")
                mscr = p_four.tile([64, 2, 64, 32], BF16, tag="xs")
                y_S = p_bigT.tile([128, 4, BL, D], BF16, tag="bigT")
                mt5 = modes_T[:].rearrange("p hp o (g bb) -> p g hp o bb",
                                           g=4)
                for gp in range(2):
                    for g in (2 * gp, 2 * gp + 1):
                        for ri in range(2):
                            for hp in range(2):
                                nc.vector.transpose(
                                    mscr[ri * 32:(ri + 1) * 32, hp, :, :]
                                    .rearrange("p o w -> p (o w)"),
                                    modes_sb[32 * g:32 * (g + 1), hp, ri,
                                             :, :]
                                    .rearrange("p o m -> p (o m)"))
                        nc.vector.tensor_copy(
                            modes_T[:, :, :, g * 8:(g + 1) * 8],
                            mscr[:, :, :, 0:8])
                    # iDFT for output cols h in {4gp..4gp+3} ((g,hp) pairs
                    # with g in this half)
                    for lc in range(4):
                        for b in range(BL):
                            ps_y = p_ps.tile([128, 256], F32, tag="ps")
                            nc.tensor.matmul(
                                ps_y[:],
                                dst_t[:, lc * 128:(lc + 1) * 128],
                                mt5[:, 2 * gp:2 * gp + 2, :, :, b],
                                start=True, stop=True)
                            nc.scalar.copy(
                                y_S[:, lc, b, gp * 256:(gp + 1) * 256],
                                ps_y[:])
                    # Wo for token tiles ht in {2gp, 2gp+1}
                    # (token l = (h,e); ht tile = h-pair = g)
                    for b in range(BL):
                        for ht in (2 * gp, 2 * gp + 1):
                            ps_wo = p_ps.tile([128, D], F32, tag="ps")
                            for jc in range(4):
                                nc.tensor.matmul(
                                    ps_wo[:],
                                    y_S[:, jc, b, ht * 128:(ht + 1) * 128],
                                    wot_t[:, jc, :],
                                    start=(jc == 0), stop=(jc == 3))
                            nc.vector.tensor_add(x_S[:, b, ht, :],
                                                 x_S[:, b, ht, :], ps_wo[:])

                # ---- decomp1 ----
                _decomp(nc, p_ps, p_xbf, x_S, adiag_t, aup_t, alo_t)

                if l == 0:
                    dbg_dump("res1", x_S[:], [128, BL, 4, D])

                # ---- res1_T ----
                for b in range(BL):
                    for c in range(4):
                        t0 = (b * 4 + c) * 128
                        nc.gpsimd.dma_start(xrow[t0:t0 + 128, :],
                                            x_S[:, b, c, :])
                r1T = p_bigT.tile([128, 4, NT], BF16, tag="bigT")
                for k in range(4):
                    nc.sync.dma_start_transpose(
                        r1T[:, k, :], xrow[:, k * 128:(k + 1) * 128])

                # ---- FFN (two f-half passes, partial z-adds) ----
                for fh in range(2):
                    c1_t = p_wly.tile([128, 4, 8, 128], BF16, tag="c1")
                    nc.sync.dma_start(
                        c1_t[:], P[f"c1{l}"][:, :, fh * 8:(fh + 1) * 8, :])
                    c2_t = p_wly.tile([128, 8, D], BF16, tag="c2")
                    nc.sync.dma_start(
                        c2_t[:], P[f"c2{l}"][:, fh * 8:(fh + 1) * 8, :])
                    for b in range(BL):
                        y1_t = p_wcb.tile([128, 8, D], BF16, tag="wcb")
                        for ft in range(8):
                            ps_f1 = p_ps.tile([128, D], F32, tag="ps")
                            for dc in range(4):
                                nc.tensor.matmul(
                                    ps_f1[:], c1_t[:, dc, ft, :],
                                    r1T[:, dc, b * 512:(b + 1) * 512],
                                    start=(dc == 0), stop=(dc == 3))
                            nc.scalar.activation(y1_t[:, ft, :], ps_f1[:],
                                                 AF.Relu)
                        for ht in range(4):
                            ps_f2 = p_ps.tile([128, D], F32, tag="ps")
                            for fc in range(8):
                                nc.tensor.matmul(
                                    ps_f2[:],
                                    y1_t[:, fc, ht * 128:(ht + 1) * 128],
                                    c2_t[:, fc, :],
                                    start=(fc == 0), stop=(fc == 7))
                            nc.vector.tensor_add(x_S[:, b, ht, :],
                                                 x_S[:, b, ht, :], ps_f2[:])

                # ---- decomp2 (last layer: pipeline LN stats per-b) ----
                if l < ELAYERS - 1:
                    _decomp(nc, p_ps, p_xbf, x_S, adiag_t, aup_t, alo_t)
                else:
                    sum_x = p_sml.tile([128, 32], F32, tag="sumx")
                    sum_sq = p_sml.tile([128, 32], F32, tag="sumsq")

                    def _ln_stats(b, x_bf):
                        nc.vector.tensor_reduce(
                            sum_x[:, 4 * b:4 * b + 4],
                            x_S[:, b, :, :], mybir.AxisListType.X, ALU.add)
                        for c in range(4):
                            # x_bf is dead after this b's decomp matmuls;
                            # reuse it as the Square dump target
                            nc.scalar.activation(
                                x_bf[:, c, :], x_S[:, b, c, :], AF.Square,
                                accum_out=sum_sq[:, 4 * b + c:4 * b + c + 1])

                    _decomp(nc, p_ps, p_xbf, x_S, adiag_t, aup_t, alo_t,
                            post_b=_ln_stats)

            dbg_dump("xfin", x_S[:], [128, BL, 4, D])

            # ---------------- final layernorm + projection ----------------
            mu = p_sml.tile([128, 32], F32, tag="mu")
            nc.vector.tensor_scalar_mul(mu[:], sum_x[:], 1.0 / D)
            var = p_sml.tile([128, 32], F32, tag="var")
            nc.vector.tensor_mul(var[:], mu[:], mu[:])
            nc.vector.scalar_tensor_tensor(
                out=var[:], in0=sum_sq[:], scalar=1.0 / D, in1=var[:],
                op0=ALU.mult, op1=ALU.subtract)
            nc.vector.tensor_scalar_add(var[:], var[:], 1e-5)
            std = p_sml.tile([128, 32], F32, tag="std")
            nc.scalar.activation(std[:], var[:], AF.Sqrt)
            rstd = p_sml.tile([128, 32], F32, tag="rstd")
            nc.vector.reciprocal(rstd[:], std[:])

            # xh0 = (x - mu) * rstd   (bf16; affine w folded at the end,
            # bias b cancels in xh[last] - mean_l(xh))
            xh0 = p_bigT.tile([128, 4, BL, D], BF16, tag="bigT")
            for j in range(32):
                b, c = j // 4, j % 4
                nc.vector.tensor_scalar(
                    xh0[:, c, b, :], x_S[:, b, c, :],
                    mu[:, j:j + 1], rstd[:, j:j + 1],
                    ALU.subtract, ALU.mult)

            last_out = p_sml.tile([8, 536], F32, tag="lastout")
            for b in range(BL):
                ps_m = p_ps.tile([1, D], F32, tag="ps")
                for c in range(4):
                    nc.tensor.matmul(ps_m[:], ones_t[:], xh0[:, c, b, :],
                                     start=(c == 0), stop=(c == 3))
                # engine APs must start at partition 0/32/64/96; move the
                # last-token row (partition 127) and the per-b output row
                # via small SBUF-to-SBUF DMAs instead.
                xlast = p_sml.tile([1, D], BF16, tag="xlast")
                nc.sync.dma_start(xlast[:], xh0[127:128, 3, b, :])
                diff = p_sml.tile([1, D], F32, tag="diff")
                nc.vector.scalar_tensor_tensor(
                    out=diff[:], in0=ps_m[:], scalar=-1.0 / L,
                    in1=xlast[:], op0=ALU.mult, op1=ALU.add)
                row = p_sml.tile([1, D], F32, tag="row")
                nc.vector.tensor_mul(row[:], diff[:], normw_t[0:1, :])
                nc.sync.dma_start(last_out[b:b + 1, 0:D], row[:])
            nc.sync.dma_start(last_out[:, D:D + 24], P["tv_last"][:])

            # transpose last_out -> lastT (bf16 for the projection matmul)
            lastT = p_sml.tile([128, 4, 8], BF16, tag="lastT")
            for ck in range(4):
                ps_t = p_ps.tile([128, 8], F32, tag="ps")
                nc.tensor.transpose(ps_t[:],
                                    last_out[:, ck * 128:(ck + 1) * 128],
                                    id8_t[:])
                nc.vector.tensor_copy(lastT[:, ck, :], ps_t[:])
            lastT4 = p_sml.tile([24, 8], BF16, tag="lastT4")
            ps_t4 = p_ps.tile([24, 8], F32, tag="ps")
            nc.tensor.transpose(ps_t4[:], last_out[:, 512:536], id8_t[:])
            nc.vector.tensor_copy(lastT4[:], ps_t4[:])

            # projection, streamed over 20 vocab slices
            for vs in range(20):
                v0 = vs * 512
                vw = min(512, NUM_APP - v0)
                pw_t = p_pw.tile([128, 4, 512], BF16, tag="pw")
                nc.sync.dma_start(pw_t[:, :, 0:vw], P["pwt"][:, :, v0:v0 + vw])
                pw4_t = p_pws.tile([24, 512], BF16, tag="pw4")
                nc.sync.dma_start(pw4_t[:, 0:vw], P["pwt4"][:, v0:v0 + vw])
                pb_t = p_pws.tile([8, 512], F32, tag="pb")
                nc.sync.dma_start(pb_t[:, 0:vw], P["pb"][:, v0:v0 + vw])
                ps_p = p_ps2.tile([8, 512], F32, tag="psxs")
                for ck in range(4):
                    nc.tensor.matmul(ps_p[:, 0:vw], lastT[:, ck, :],
                                     pw_t[:, ck, 0:vw],
                                     start=(ck == 0), stop=False)
                nc.tensor.matmul(ps_p[:, 0:vw], lastT4[:], pw4_t[:, 0:vw],
                                 start=False, stop=True)
                sc_t = p_pws.tile([8, 512], F32, tag="sc")
                nc.vector.tensor_add(sc_t[:, 0:vw], ps_p[:, 0:vw],
                                     pb_t[:, 0:vw])
                nc.sync.dma_start(OUT[:, v0:v0 + vw], sc_t[:, 0:vw])

    return nc


# ---------------------------------------------------------------- runner

_CACHED = {}


def kernel(**inputs):
    install()

    x_app = np.asarray(inputs["x_app"])
    x_time = np.asarray(inputs["x_time"], np.float32)
    time_vecs = np.asarray(inputs["time_vecs"], np.float32)

    w = prep_weights(inputs)
    if "nc" not in _CACHED:
        _CACHED["nc"] = build_nc()
    nc = _CACHED["nc"]

    in_maps = []
    for core in range(NCORES):
        sl = slice(core * BL, (core + 1) * BL)
        m = {"x_app": np.ascontiguousarray(x_app[sl]).astype(np.int32),
             "x_time": np.ascontiguousarray(x_time[sl]),
             "tv_last": np.ascontiguousarray(time_vecs[sl, L - 1, :])}
        m.update(w)
        in_maps.append(m)

    res = run_bass_kernel_spmd(nc, in_maps, list(range(NCORES)))
    out = np.concatenate([res.results[i]["out"] for i in range(NCORES)],
                         axis=0)
    return out.astype(np.float32)


if __name__ == "__main__":
    import reference
    inp = {k: np.asarray(v) for k, v in reference.setup_inputs().items()}
    got = kernel(**inp)
    exp = np.asarray(reference.reference(**reference.setup_inputs()))
    err = np.linalg.norm(got - exp) / np.linalg.norm(exp)
    print("Relative error:", err)



# revision 31
# speedup vs baseline: 1.1684x; 1.1684x over previous
"""AppUsageFEDformer Trainium2 kernel — 8-core data-parallel Bass implementation.

Strategy: pure data parallelism over batch (64 -> 8 per NeuronCore).  Each
core runs the full model on its batch shard:
  embedding gather -> 2x encoder layer (Q-proj, Fourier block via DFT
  matmuls, mode mix, iDFT, out-proj, series-decomp as banded matmul, FFN)
  -> final my_layernorm -> vocab projection.

v2: fp16 spine + fp16 matmuls everywhere (better than bf16 numerically and
2-byte DVE fast modes), series-decomp folded to (I-M) reading the spine
directly, per-batch pipelined embedding/bounce/transpose chains, merged
PSUM->SBUF copies, hoisted memsets, prefetched projection weights.

Layouts per core (b = local batch 0..7, l = seq 0..511, tokens t = b*512+l):
  x_S  (spine, fp16):  SBUF [128(l%128), (b, c=l//128, d)]  "S layout"
  x_T  (fp16):         SBUF [128(d%128), (dc=d//128, t)]    "T layout",
                       built via DMA to DRAM + xbar DMA-transpose (per-b).
All matmuls run in fp16 (f32 PSUM accumulate).
"""

import os

import numpy as np

import concourse.bass as bass
import concourse.tile as tile
from concourse import mybir
from concourse.bass_utils import run_bass_kernel_spmd


# ------------------------------------------------------------------
# BIR wait-legalizer (inlined): the axon-path walrus rejects
# instructions carrying more than one sem wait ("Too many sync wait
# commands").  Hoist excess waits onto injected same-engine Drains.

import json

_CAP = 1  # max waits left on any real instruction


def _mk_nop(engine, name, waits):
    return {
        "debug": 0,
        "engine": engine,
        "ins": [],
        "name": name,
        "opcode": "Drain",
        "outs": [],
        "sync_info": {"on_update": [], "on_wait": waits},
    }


def legalize_bir_waits(bir_json: bytes) -> bytes:
    d = json.loads(bir_json)
    ctr = [0]
    changed = [0]
    for fn in d.get("functions", []):
        for bb in fn.get("blocks", []):
            insts = bb.get("instructions")
            if not insts:
                continue
            out = []
            for inst in insts:
                si = inst.get("sync_info")
                waits = (si or {}).get("on_wait") or []
                if len(waits) > _CAP:
                    keep = waits[:_CAP]
                    excess = waits[_CAP:]
                    for w in excess:
                        ctr[0] += 1
                        out.append(_mk_nop(inst["engine"], f"I-nopw{ctr[0]}", [w]))
                    si["on_wait"] = keep
                    changed[0] += 1
                out.append(inst)
            bb["instructions"] = out
    return json.dumps(d).encode()


def install():
    """Patch concourse.bass2jax.compile_bir_kernel to legalize first."""
    import concourse.bass2jax as b2j
    if getattr(b2j, "_legalize_installed", False):
        return
    orig = b2j.compile_bir_kernel

    def wrapped(bir_json, tmpdir, neff_name="file.neff"):
        return orig(legalize_bir_waits(bir_json), tmpdir, neff_name)

    b2j.compile_bir_kernel = wrapped
    b2j._legalize_installed = True


F32 = mybir.dt.float32
F16 = mybir.dt.float16
I32 = mybir.dt.int32
AF = mybir.ActivationFunctionType
ALU = mybir.AluOpType

B, L, D, H, DFF, MODES, ELAYERS = 64, 512, 512, 8, 2048, 32, 2
VOCAB, NUM_APP, KERNEL = 10000, 10000, 25
E = D // H  # 64
NCORES = 8
BL = B // NCORES  # 8 local batch
NT = BL * L       # 4096 local tokens
f16 = np.float16


# ---------------------------------------------------------------- host prep

def _movavg_matrix():
    """M[l_src, l_out]: weight of x[l_src] in moving_avg[l_out], including
    edge replication (pad (K-1)//2 each side with edge values)."""
    M = np.zeros((L, L), np.float64)
    pad = (KERNEL - 1) // 2
    for lo in range(L):
        for j in range(lo - pad, lo + pad + 1):
            M[min(max(j, 0), L - 1), lo] += 1.0 / KERNEL
    return M.astype(np.float32)


def _dft_c4():
    """C4 [128, (4 c, 64)]: stationary for DFT.  col j<32: cos(2pi*l*j/L);
    j>=32: -sin(2pi*l*(j-32)/L) with l = c*128+p."""
    out = np.zeros((128, 4, 64), np.float32)
    for c in range(4):
        lv = c * 128 + np.arange(128)
        for m in range(MODES):
            ang = 2.0 * np.pi * lv * m / L
            out[:, c, m] = np.cos(ang)
            out[:, c, 32 + m] = -np.sin(ang)
    return out.astype(f16)


def _idft_d():
    """Dstack [64, 512]: rows m<32: sc(m)*cos(2pi*m*l'/L); rows 32+m:
    -sc(m)*sin(...), sc = (2-delta_m0)/L."""
    out = np.zeros((64, L), np.float32)
    lp = np.arange(L)
    for m in range(MODES):
        sc = (1.0 if m == 0 else 2.0) / L
        ang = 2.0 * np.pi * m * lp / L
        out[m] = sc * np.cos(ang)
        out[32 + m] = -sc * np.sin(ang)
    return out.astype(f16)


def prep_weights(inp):
    """Pre-arrange all weights into SBUF-shaped host arrays (fp16)."""
    w = {}
    w["embt"] = (np.asarray(inp["app_emb_w"], np.float32)
                 + np.asarray(inp["time_b"], np.float32)[None, :]
                 ).astype(f16)
    w["tw_rep"] = np.broadcast_to(
        np.asarray(inp["time_w"], np.float32), (128, D)).astype(f16).copy()
    w["c4"] = _dft_c4()
    w["dstack"] = _idft_d()

    # A' = I - M folded decomp: res = A'^T-ish banded matmul of x.
    M = _movavg_matrix()
    A = np.eye(L, dtype=np.float32) - M
    adiag = np.zeros((128, 4, 128), np.float32)
    for c in range(4):
        adiag[:, c, :] = A[c * 128:(c + 1) * 128, c * 128:(c + 1) * 128]
    w["adiag"] = adiag.astype(f16)
    # band tiles zero-padded to 32/64-aligned partition bases (matmul
    # requires base_partition in {0, 32, 64} matching on both operands)
    aup = np.zeros((32, 3, 128), np.float32)    # tile (co+1, co): rows 0:12
    alo = np.zeros((128, 3, 128), np.float32)   # tile (co-1, co): rows 116:128
    for co in range(3):
        aup[0:12, co, :] = A[(co + 1) * 128:(co + 1) * 128 + 12,
                             co * 128:(co + 1) * 128]
    for co in range(1, 4):
        alo[116:128, co - 1, :] = A[co * 128 - 12:co * 128,
                                    co * 128:(co + 1) * 128]
    w["aup"] = aup.astype(f16)
    w["alo"] = alo.astype(f16)

    Wq = np.asarray(inp["Wq"], np.float32)
    Wo = np.asarray(inp["Wo"], np.float32)
    wr = np.asarray(inp["four_wr"], np.float32)
    wi = np.asarray(inp["four_wi"], np.float32)
    c1 = np.asarray(inp["conv1_w"], np.float32)
    c2 = np.asarray(inp["conv2_w"], np.float32)
    bq = np.asarray(inp["bq"], np.float32)

    for l in range(ELAYERS):
        wqt = np.zeros((128, 4, D), np.float32)
        for k in range(4):
            wqt[:, k, :] = Wq[l].T[k * 128:(k + 1) * 128, :]
        w[f"wqt{l}"] = wqt.astype(f16)
        # bq is folded into the DFT output bias: xs += sum_l C[l,m] * bq
        # (only the m=0 cos row survives: 512*bq).  bo cancels exactly in
        # series_decomp ((I-M) of a constant is 0), so the +bo add is dropped.
        xs_bias = np.zeros((64, 2, H, 32), np.float32)   # [m_ri,(q,h,i32)]
        xs_bias[0, :, :, :] = (512.0 * bq[l].reshape(H, 2, 32)
                               ).transpose(1, 0, 2)
        w[f"xsb{l}"] = xs_bias

        # m-chunk outermost so each per-mh DMA slice is contiguous
        wc = np.zeros((128, 8, H, 4, 128), np.float32)
        for h in range(H):
            for m in range(MODES):
                mh, mi = m // 4, m % 4
                wc[0:64, mh, h, mi, 0:64] = wr[l, h, :, :, m]     # i x o
                wc[0:64, mh, h, mi, 64:128] = wi[l, h, :, :, m]
                wc[64:128, mh, h, mi, 0:64] = -wi[l, h, :, :, m]
                wc[64:128, mh, h, mi, 64:128] = wr[l, h, :, :, m]
        w[f"wc{l}"] = wc.astype(f16)

        wot = np.zeros((128, 4, D), np.float32)
        for jc in range(4):
            wot[:, jc, :] = Wo[l].T[jc * 128:(jc + 1) * 128, :]
        w[f"wot{l}"] = wot.astype(f16)

        # f-half outermost so each per-fh DMA slice is contiguous
        c1s = np.zeros((128, 2, 4, 8, 128), np.float32)
        for dc in range(4):
            for ft in range(16):
                c1s[:, ft // 8, dc, ft % 8, :] = \
                    c1[l][ft * 128:(ft + 1) * 128,
                          dc * 128:(dc + 1) * 128].T
        w[f"c1{l}"] = c1s.astype(f16)

        c2t = np.zeros((128, 16, D), np.float32)
        for fc in range(16):
            c2t[:, fc, :] = c2[l].T[fc * 128:(fc + 1) * 128, :]
        w[f"c2{l}"] = c2t.astype(f16)

    w["normw"] = np.broadcast_to(
        np.asarray(inp["norm_w"], np.float32), (128, D)).copy()
    pw = np.asarray(inp["proj_w"], np.float32)       # [10000, 536]
    pwt = np.zeros((128, 4, NUM_APP), np.float32)
    for ck in range(4):
        pwt[:, ck, :] = pw.T[ck * 128:(ck + 1) * 128, :]
    w["pwt"] = pwt.astype(f16)
    w["pwt4"] = pw.T[512:536, :].astype(f16)         # [24, 10000]
    w["pb"] = np.broadcast_to(
        np.asarray(inp["proj_b"], np.float32), (8, NUM_APP)).astype(f16).copy()
    w["ones"] = np.ones((128, 1), np.float32).astype(f16)
    w["ident8"] = np.eye(8, dtype=np.float32)
    return w


# ---------------------------------------------------------------- builder

def build_nc(num_devices=NCORES, debug=False):
    nc = bass.Bass("TRN2", target_bir_lowering=False, debug=False,
                   num_devices=num_devices)
    P = {}

    def param(name, shape, dtype):
        P[name] = nc.declare_dram_parameter(name, list(shape), dtype,
                                            isOutput=False)

    param("x_app", [BL, L], I32)
    param("x_time", [BL, L], F32)
    param("tv_last", [BL, 24], F32)
    param("embt", [VOCAB, D], F16)
    param("tw_rep", [128, D], F16)
    param("c4", [128, 4, 64], F16)
    param("dstack", [64, L], F16)
    param("adiag", [128, 4, 128], F16)
    param("aup", [32, 3, 128], F16)
    param("alo", [128, 3, 128], F16)
    for l in range(ELAYERS):
        param(f"wqt{l}", [128, 4, D], F16)
        param(f"xsb{l}", [64, 2, H, 32], F32)
        param(f"wc{l}", [128, 8, H, 4, 128], F16)
        param(f"wot{l}", [128, 4, D], F16)
        param(f"c1{l}", [128, 2, 4, 8, 128], F16)
        param(f"c2{l}", [128, 16, D], F16)
    param("normw", [128, D], F32)
    param("pwt", [128, 4, NUM_APP], F16)
    param("pwt4", [24, NUM_APP], F16)
    param("pb", [8, NUM_APP], F16)
    param("ones", [128, 1], F16)
    param("ident8", [8, 8], F32)

    OUT = nc.declare_dram_parameter("out", [BL, NUM_APP], F32, isOutput=True)
    xrow = nc.dram_tensor("xrow", [NT, D], F16)   # transpose bounce

    with tile.TileContext(nc) as tc:
        with tc.tile_pool(name="spine", bufs=1) as p_spine, \
             tc.tile_pool(name="bigT", bufs=1) as p_bigT, \
             tc.tile_pool(name="qsb", bufs=2) as p_qsb, \
             tc.tile_pool(name="four", bufs=1) as p_four, \
             tc.tile_pool(name="wcb", bufs=2) as p_wcb, \
             tc.tile_pool(name="y1p", bufs=1) as p_y1, \
             tc.tile_pool(name="wts", bufs=1) as p_wts, \
             tc.tile_pool(name="wly", bufs=1) as p_wly, \
             tc.tile_pool(name="ffw", bufs=2) as p_ffw, \
             tc.tile_pool(name="sml", bufs=1) as p_sml, \
             tc.tile_pool(name="pwp", bufs=2) as p_pw, \
             tc.tile_pool(name="pws", bufs=2) as p_pws, \
             tc.tile_pool(name="ps", bufs=3, space="PSUM") as p_ps, \
             tc.tile_pool(name="psd", bufs=3, space="PSUM") as p_psd, \
             tc.tile_pool(name="ps2", bufs=2, space="PSUM") as p_ps2:

            # ---------------- constants / shared weights ----------------
            c4_t = p_wts.tile([128, 4, 64], F16, tag="c4")
            nc.sync.dma_start(c4_t[:], P["c4"][:])
            dst_t = p_wts.tile([64, L], F16, tag="dstack")
            nc.sync.dma_start(dst_t[:], P["dstack"][:])
            adiag_t = p_wts.tile([128, 4, 128], F16, tag="adiag")
            nc.sync.dma_start(adiag_t[:], P["adiag"][:])
            aup_t = p_wts.tile([32, 3, 128], F16, tag="aup")
            nc.sync.dma_start(aup_t[:], P["aup"][:])
            alo_t = p_wts.tile([128, 3, 128], F16, tag="alo")
            nc.sync.dma_start(alo_t[:], P["alo"][:])
            tw_t = p_wts.tile([128, D], F16, tag="twrep")
            nc.sync.dma_start(tw_t[:], P["tw_rep"][:])
            ones_t = p_wts.tile([128, 1], F16, tag="ones")
            nc.sync.dma_start(ones_t[:], P["ones"][:])
            id8_t = p_wts.tile([8, 8], F32, tag="id8")
            nc.sync.dma_start(id8_t[:], P["ident8"][:])
            normw_t = p_wts.tile([128, D], F32, tag="normw")
            nc.sync.dma_start(normw_t[:], P["normw"][:])

            idx_sb = p_sml.tile([128, 32], I32, tag="idxsb")
            # idx_sb[p, b*4+c] = x_app[b, c*128+p]  (= token t's app id)
            nc.sync.dma_start(
                idx_sb[:],
                P["x_app"].ap().rearrange("b (c p) -> p (b c)", p=128))
            xt_t = p_sml.tile([128, 32], F32, tag="xt")
            nc.sync.dma_start(
                xt_t[:],
                P["x_time"].ap().rearrange("b (c p) -> p (b c)", p=128))

            x_S = p_spine.tile([128, BL, 4, D], F16, tag="spine")

            modes_sb = p_four.tile([128, 2, 2, 64, MODES], F16, tag="modes")
            xsT = p_four.tile([128, BL, H, 32], F16, tag="xsT")

            sum_x = p_sml.tile([128, 32], F32, tag="sumx")
            sum_sq = p_sml.tile([128, 32], F32, tag="sumsq")
            sqscr = p_sml.tile([128, D], F16, tag="sqscr")

            # prefetch first projection-weight slices (no deps; loads early
            # and overlaps the layers)
            VS = 20
            pw_tiles = {}
            for vs in range(2):
                pw_tiles[vs] = p_pw.tile([128, 4, 512], F16, tag="pw",
                                         name=f"pwpre{vs}")
                nc.sync.dma_start(pw_tiles[vs][:, :, 0:512],
                                  P["pwt"][:, :, vs * 512:(vs + 1) * 512])

            def bounce_b(b):
                """x_S[:, b] -> DRAM rows -> x_T-style transpose columns."""
                nc.sync.dma_start(
                    xrow.ap()[b * 512:(b + 1) * 512, :]
                    .rearrange("(c p) d -> p c d", p=128),
                    x_S[:, b, :, :])

            def transpose_b(dst, b):
                for k in range(4):
                    nc.sync.dma_start_transpose(
                        dst[:, k, b * 512:(b + 1) * 512],
                        xrow.ap()[b * 512:(b + 1) * 512,
                                  k * 128:(k + 1) * 128])

            def decomp_b(nc, b):
                """x_S[:, b] <- (I-M) applied over l to x_S[:, b].
                All matmuls read the original x_S[:, b] (program order), the
                copies write the banded-transform result back."""
                pss = []
                for co in range(4):
                    src = x_S[:, b, :, :]
                    ps_a = p_psd.tile([128, D], F32, tag="psd")
                    mms = [(adiag_t[:, co, :], src[:, co, :])]
                    if co < 3:
                        mms.append((aup_t[:, co, :], src[0:32, co + 1, :]))
                    if co > 0:
                        mms.append((alo_t[64:128, co - 1, :],
                                    src[64:128, co - 1, :]))
                    for i, (lhsT, rhs) in enumerate(mms):
                        nc.tensor.matmul(ps_a[:], lhsT, rhs,
                                         start=(i == 0),
                                         stop=(i == len(mms) - 1))
                    pss.append(ps_a)
                for co in range(4):
                    nc.vector.tensor_copy(x_S[:, b, co, :], pss[co][:])

            # ---------------- layers ----------------
            for l in range(ELAYERS):
                wqt_t = p_wly.tile([128, 4, D], F16, tag="wqt")
                nc.sync.dma_start(wqt_t[:], P[f"wqt{l}"][:])
                xsb_t = p_wly.tile([64, 2, H, 32], F32, tag="xsb")
                nc.sync.dma_start(xsb_t[:], P[f"xsb{l}"][:])
                wot_t = p_wly.tile([128, 4, D], F16, tag="wot")
                nc.sync.dma_start(wot_t[:], P[f"wot{l}"][:])

                # ---- per-b: (emb) -> bounce -> transpose -> Q-proj ->
                #      DFT -> shuffle1 ----
                x_T = p_bigT.tile([128, 4, NT], F16, tag="bigT")
                for b in range(BL):
                    if l == 0:
                        for c in range(4):
                            j = b * 4 + c
                            nc.gpsimd.indirect_dma_start(
                                out=x_S[:, b, c, :], out_offset=None,
                                in_=P["embt"][:],
                                in_offset=bass.IndirectOffsetOnAxis(
                                    ap=idx_sb[:, j:j + 1], axis=0))
                        for c in range(4):
                            j = b * 4 + c
                            nc.vector.scalar_tensor_tensor(
                                out=x_S[:, b, c, :], in0=tw_t[:],
                                scalar=xt_t[:, j:j + 1], in1=x_S[:, b, c, :],
                                op0=ALU.mult, op1=ALU.add)
                    bounce_b(b)
                    transpose_b(x_T, b)

                    q_sb = p_qsb.tile([128, 4, D], F16, tag="qsb")
                    for c in range(4):
                        t0 = (b * 4 + c) * 128
                        ps_q = p_ps.tile([128, D], F32, tag="ps")
                        for k in range(4):
                            nc.tensor.matmul(
                                ps_q[:], x_T[:, k, t0:t0 + 128],
                                wqt_t[:, k, :],
                                start=(k == 0), stop=(k == 3))
                        nc.scalar.copy(q_sb[:, c, :], ps_q[:])
                    ps_xs = p_ps2.tile([64, D], F32, tag="psxs")
                    for c in range(4):
                        nc.tensor.matmul(ps_xs[:], c4_t[:, c, :],
                                         q_sb[:, c, :],
                                         start=(c == 0), stop=(c == 3))
                    # psum free order (h, q, i32) -> xs_b[:, q, h, i32]
                    xs_b = p_qsb.tile([64, 2, H, 32], F16, tag="xsb")
                    nc.vector.tensor_add(
                        xs_b[:].rearrange("p q h i -> p h q i"),
                        ps_xs[:].rearrange("p (h q i) -> p h q i", h=8, q=2),
                        xsb_t[:].rearrange("p q h i -> p h q i"))

                    # shuffle1: xs -> xsT [(ri,i), (b, h, m)]
                    for ri in range(2):
                        for q in range(2):
                            r0 = ri * 64 + q * 32
                            nc.vector.transpose(
                                xsT[r0:r0 + 32, b, :, :]
                                .rearrange("p h m -> p (h m)"),
                                xs_b[ri * 32:ri * 32 + 32, q, :, :]
                                .rearrange("p h i -> p (h i)"))

                # ---- mode mix ----
                # psum tile (mh, hp): [128, 512]; rows 32g+b (g = h//2);
                # cols mi*128 + (ri*64 + o).  modes_sb rows sparse 32g+b.
                for mh in range(8):
                    m0 = mh * 4
                    wc_t = p_wcb.tile([128, H, 4, 128], F16, tag="wcb")
                    nc.sync.dma_start(wc_t[:], P[f"wc{l}"][:, mh])
                    for hp in range(2):
                        ps_mm = p_ps.tile([128, D], F32, tag="ps")
                        # zero rows 32g+8..32g+32 that no matmul writes, so
                        # the merged copy below reads no PSUM garbage
                        # (GPSIMD cannot access PSUM -> DVE)
                        nc.vector.memset(ps_mm[:], 0.0)
                        for g in range(4):
                            h = g * 2 + hp
                            for mi in range(4):
                                nc.tensor.matmul(
                                    ps_mm[32 * g:32 * g + 8,
                                          mi * 128:(mi + 1) * 128],
                                    xsT[:, :, h, m0 + mi],
                                    wc_t[:, h, mi, :],
                                    start=True, stop=True,
                                    tile_position=(0, 32 * g))
                        # single merged copy for all 4 g-blocks (rows 32g+bb;
                        # rows 8..32 of each block stay zero from the memset)
                        nc.scalar.copy(
                            modes_sb[:, hp, :, :, m0:m0 + 4]
                            .rearrange("p r o m -> p m r o"),
                            ps_mm[:].rearrange("p (m r o) -> p m r o",
                                               m=4, r=2))

                # ---- shuffle2 / iDFT / Wo, pipelined in g-pair halves:
                # the DVE stream-transposes of half gp overlap the PE
                # iDFT+Wo matmuls of half gp-1 ----
                modes_T = p_four.tile([64, 2, 64, 32], F16, tag="modesT")
                mscr = p_four.tile([64, 2, 64, 32], F16, tag="mscr")
                y_S = p_bigT.tile([128, 4, BL, D], F16, tag="bigT")
                mt5 = modes_T[:].rearrange("p hp o (g bb) -> p g hp o bb",
                                           g=4)
                for gp in range(2):
                    for g in (2 * gp, 2 * gp + 1):
                        for ri in range(2):
                            for hp in range(2):
                                nc.vector.transpose(
                                    mscr[ri * 32:(ri + 1) * 32, hp, :, :]
                                    .rearrange("p o w -> p (o w)"),
                                    modes_sb[32 * g:32 * (g + 1), hp, ri,
                                             :, :]
                                    .rearrange("p o m -> p (o m)"))
                        nc.vector.tensor_copy(
                            modes_T[:, :, :, g * 8:(g + 1) * 8],
                            mscr[:, :, :, 0:8])
                    # iDFT for output cols h in {4gp..4gp+3}, two b at a time
                    for lc in range(4):
                        for bp in range(4):
                            ps_y = p_ps.tile([128, 512], F32, tag="ps")
                            nc.tensor.matmul(
                                ps_y[:],
                                dst_t[:, lc * 128:(lc + 1) * 128],
                                mt5[:, 2 * gp:2 * gp + 2, :, :,
                                    2 * bp:2 * bp + 2],
                                start=True, stop=True)
                            # psum cols (g2, hp, o, b2) -> y_S d-cols per b
                            nc.scalar.copy(
                                y_S[:, lc, 2 * bp:2 * bp + 2,
                                    gp * 256:(gp + 1) * 256]
                                .rearrange("p b d -> p d b"),
                                ps_y[:].rearrange("p (d b) -> p d b", b=2))
                    # Wo for token tiles ht in {2gp, 2gp+1}
                    # (token l = (h,e); ht tile = h-pair = g)
                    for b in range(BL):
                        for ht in (2 * gp, 2 * gp + 1):
                            ps_wo = p_ps.tile([128, D], F32, tag="ps")
                            for jc in range(4):
                                nc.tensor.matmul(
                                    ps_wo[:],
                                    y_S[:, jc, b, ht * 128:(ht + 1) * 128],
                                    wot_t[:, jc, :],
                                    start=(jc == 0), stop=(jc == 3))
                            nc.vector.tensor_add(x_S[:, b, ht, :],
                                                 x_S[:, b, ht, :], ps_wo[:])

                # ---- decomp1 + bounce for FFN (per b) ----
                r1T = p_bigT.tile([128, 4, NT], F16, tag="bigT")
                for b in range(BL):
                    decomp_b(nc, b)
                    bounce_b(b)
                    transpose_b(r1T, b)

                # ---- FFN (two f-half passes, partial z-adds) ----
                for fh in range(2):
                    c1_t = p_ffw.tile([128, 4, 8, 128], F16, tag="c1")
                    nc.sync.dma_start(c1_t[:], P[f"c1{l}"][:, fh])
                    c2_t = p_ffw.tile([128, 8, D], F16, tag="c2")
                    nc.sync.dma_start(
                        c2_t[:], P[f"c2{l}"][:, fh * 8:(fh + 1) * 8, :])
                    for b in range(BL):
                        y1_t = p_y1.tile([128, 8, D], F16, tag="y1")
                        for ft in range(8):
                            ps_f1 = p_ps.tile([128, D], F32, tag="ps")
                            for dc in range(4):
                                nc.tensor.matmul(
                                    ps_f1[:], c1_t[:, dc, ft, :],
                                    r1T[:, dc, b * 512:(b + 1) * 512],
                                    start=(dc == 0), stop=(dc == 3))
                            nc.scalar.activation(y1_t[:, ft, :], ps_f1[:],
                                                 AF.Relu)
                        for ht in range(4):
                            ps_f2 = p_ps.tile([128, D], F32, tag="ps")
                            for fc in range(8):
                                nc.tensor.matmul(
                                    ps_f2[:],
                                    y1_t[:, fc, ht * 128:(ht + 1) * 128],
                                    c2_t[:, fc, :],
                                    start=(fc == 0), stop=(fc == 7))
                            nc.vector.tensor_add(x_S[:, b, ht, :],
                                                 x_S[:, b, ht, :], ps_f2[:])

                # ---- decomp2 (last layer: pipeline LN stats per-b) ----
                if l < ELAYERS - 1:
                    for b in range(BL):
                        decomp_b(nc, b)
                else:
                    for b in range(BL):
                        decomp_b(nc, b)
                        nc.vector.tensor_reduce(
                            sum_x[:, 4 * b:4 * b + 4],
                            x_S[:, b, :, :], mybir.AxisListType.X, ALU.add)
                        for c in range(4):
                            nc.scalar.activation(
                                sqscr[:], x_S[:, b, c, :], AF.Square,
                                accum_out=sum_sq[:, 4 * b + c:4 * b + c + 1])

            # ---------------- final layernorm + projection ----------------
            mu = p_sml.tile([128, 32], F32, tag="mu")
            nc.vector.tensor_scalar_mul(mu[:], sum_x[:], 1.0 / D)
            var = p_sml.tile([128, 32], F32, tag="var")
            nc.vector.tensor_mul(var[:], mu[:], mu[:])
            nc.vector.scalar_tensor_tensor(
                out=var[:], in0=sum_sq[:], scalar=1.0 / D, in1=var[:],
                op0=ALU.mult, op1=ALU.subtract)
            nc.vector.tensor_scalar_add(var[:], var[:], 1e-5)
            std = p_sml.tile([128, 32], F32, tag="std")
            nc.scalar.activation(std[:], var[:], AF.Sqrt)
            rstd = p_sml.tile([128, 32], F32, tag="rstd")
            nc.vector.reciprocal(rstd[:], std[:])

            # xh0 = (x - mu) * rstd   (fp16; affine w folded at the end,
            # bias b cancels in xh[last] - mean_l(xh))
            xh0 = p_bigT.tile([128, 4, BL, D], F16, tag="bigT")
            for j in range(32):
                b, c = j // 4, j % 4
                nc.vector.tensor_scalar(
                    xh0[:, c, b, :], x_S[:, b, c, :],
                    mu[:, j:j + 1], rstd[:, j:j + 1],
                    ALU.subtract, ALU.mult)

            last_out = p_sml.tile([8, 536], F32, tag="lastout")
            for b in range(BL):
                ps_m = p_ps.tile([1, D], F32, tag="ps")
                for c in range(4):
                    nc.tensor.matmul(ps_m[:], ones_t[:], xh0[:, c, b, :],
                                     start=(c == 0), stop=(c == 3))
                # engine APs must start at partition 0/32/64/96; move the
                # last-token row (partition 127) and the per-b output row
                # via small SBUF-to-SBUF DMAs instead.
                xlast = p_sml.tile([1, D], F16, tag="xlast")
                nc.sync.dma_start(xlast[:], xh0[127:128, 3, b, :])
                diff = p_sml.tile([1, D], F32, tag="diff")
                nc.vector.scalar_tensor_tensor(
                    out=diff[:], in0=ps_m[:], scalar=-1.0 / L,
                    in1=xlast[:], op0=ALU.mult, op1=ALU.add)
                nc.vector.tensor_mul(diff[:], diff[:], normw_t[0:1, :])
                nc.sync.dma_start(last_out[b:b + 1, 0:D], diff[:])
            nc.sync.dma_start(last_out[:, D:D + 24], P["tv_last"][:])

            # transpose last_out -> lastT (fp16 for the projection matmul)
            lastT = p_sml.tile([128, 4, 8], F16, tag="lastT")
            for ck in range(4):
                ps_t = p_ps.tile([128, 8], F32, tag="ps")
                nc.tensor.transpose(ps_t[:],
                                    last_out[:, ck * 128:(ck + 1) * 128],
                                    id8_t[:])
                nc.vector.tensor_copy(lastT[:, ck, :], ps_t[:])
            lastT4 = p_sml.tile([24, 8], F16, tag="lastT4")
            ps_t4 = p_ps.tile([24, 8], F32, tag="ps")
            nc.tensor.transpose(ps_t4[:], last_out[:, 512:536], id8_t[:])
            nc.vector.tensor_copy(lastT4[:], ps_t4[:])

            # projection, streamed over 20 vocab slices (first 4 prefetched)
            for vs in range(VS):
                v0 = vs * 512
                vw = min(512, NUM_APP - v0)
                if vs in pw_tiles:
                    pw_t = pw_tiles[vs]
                else:
                    pw_t = p_pw.tile([128, 4, 512], F16, tag="pw")
                    nc.sync.dma_start(pw_t[:, :, 0:vw],
                                      P["pwt"][:, :, v0:v0 + vw])
                pw4_t = p_pws.tile([24, 512], F16, tag="pw4")
                nc.sync.dma_start(pw4_t[:, 0:vw], P["pwt4"][:, v0:v0 + vw])
                pb_t = p_pws.tile([8, 512], F16, tag="pb")
                nc.sync.dma_start(pb_t[:, 0:vw], P["pb"][:, v0:v0 + vw])
                ps_p = p_ps2.tile([8, 512], F32, tag="psxs")
                for ck in range(4):
                    nc.tensor.matmul(ps_p[:, 0:vw], lastT[:, ck, :],
                                     pw_t[:, ck, 0:vw],
                                     start=(ck == 0), stop=False)
                nc.tensor.matmul(ps_p[:, 0:vw], lastT4[:], pw4_t[:, 0:vw],
                                 start=False, stop=True)
                sc_t = p_pws.tile([8, 512], F32, tag="sc")
                nc.vector.tensor_add(sc_t[:, 0:vw], ps_p[:, 0:vw],
                                     pb_t[:, 0:vw])
                nc.sync.dma_start(OUT[:, v0:v0 + vw], sc_t[:, 0:vw])

    return nc


# ---------------------------------------------------------------- runner

_CACHED = {}


def kernel(**inputs):
    install()

    x_app = np.asarray(inputs["x_app"])
    x_time = np.asarray(inputs["x_time"], np.float32)
    time_vecs = np.asarray(inputs["time_vecs"], np.float32)

    w = prep_weights(inputs)
    if "nc" not in _CACHED:
        _CACHED["nc"] = build_nc()
    nc = _CACHED["nc"]

    in_maps = []
    for core in range(NCORES):
        sl = slice(core * BL, (core + 1) * BL)
        m = {"x_app": np.ascontiguousarray(x_app[sl]).astype(np.int32),
             "x_time": np.ascontiguousarray(x_time[sl]),
             "tv_last": np.ascontiguousarray(time_vecs[sl, L - 1, :])}
        m.update(w)
        in_maps.append(m)

    res = run_bass_kernel_spmd(nc, in_maps, list(range(NCORES)))
    out = np.concatenate([res.results[i]["out"] for i in range(NCORES)],
                         axis=0)
    return out.astype(np.float32)


if __name__ == "__main__":
    import reference
    inp = {k: np.asarray(v) for k, v in reference.setup_inputs().items()}
    got = kernel(**inp)
    exp = np.asarray(reference.reference(**reference.setup_inputs()))
    err = np.linalg.norm(got - exp) / np.linalg.norm(exp)
    print("Relative error:", err)


# revision 65
# speedup vs baseline: 1.2328x; 1.0551x over previous
"""AppUsageFEDformer Trainium2 kernel — 8-core data-parallel Bass implementation.

Strategy: pure data parallelism over batch (64 -> 8 per NeuronCore).  Each
core runs the full model on its batch shard:
  embedding gather -> 2x encoder layer (Q-proj, Fourier block via DFT
  matmuls, mode mix, iDFT, out-proj, series-decomp as banded matmul, FFN)
  -> final my_layernorm -> vocab projection.

v2: fp16 spine + fp16 matmuls everywhere (better than bf16 numerically and
2-byte DVE fast modes), series-decomp folded to (I-M) reading the spine
directly, per-batch pipelined embedding/bounce/transpose chains, merged
PSUM->SBUF copies, hoisted memsets, prefetched projection weights.

Layouts per core (b = local batch 0..7, l = seq 0..511, tokens t = b*512+l):
  x_S  (spine, fp16):  SBUF [128(l%128), (b, c=l//128, d)]  "S layout"
  x_T  (fp16):         SBUF [128(d%128), (dc=d//128, t)]    "T layout",
                       built via DMA to DRAM + xbar DMA-transpose (per-b).
All matmuls run in fp16 (f32 PSUM accumulate).
"""

import os

import numpy as np

import concourse.bass as bass
import concourse.tile as tile
from concourse import mybir
from concourse.bass_utils import run_bass_kernel_spmd


# ------------------------------------------------------------------
# BIR wait-legalizer (inlined): the axon-path walrus rejects
# instructions carrying more than one sem wait ("Too many sync wait
# commands").  Hoist excess waits onto injected same-engine Drains.

import json

_CAP = 1  # max waits left on any real instruction


def _mk_nop(engine, name, waits):
    return {
        "debug": 0,
        "engine": engine,
        "ins": [],
        "name": name,
        "opcode": "Drain",
        "outs": [],
        "sync_info": {"on_update": [], "on_wait": waits},
    }


def legalize_bir_waits(bir_json: bytes) -> bytes:
    d = json.loads(bir_json)
    ctr = [0]
    changed = [0]
    for fn in d.get("functions", []):
        for bb in fn.get("blocks", []):
            insts = bb.get("instructions")
            if not insts:
                continue
            out = []
            for inst in insts:
                si = inst.get("sync_info")
                waits = (si or {}).get("on_wait") or []
                if len(waits) > _CAP:
                    keep = waits[:_CAP]
                    excess = waits[_CAP:]
                    for w in excess:
                        ctr[0] += 1
                        out.append(_mk_nop(inst["engine"], f"I-nopw{ctr[0]}", [w]))
                    si["on_wait"] = keep
                    changed[0] += 1
                out.append(inst)
            bb["instructions"] = out
    return json.dumps(d).encode()


def install():
    """Patch concourse.bass2jax.compile_bir_kernel to legalize first."""
    import concourse.bass2jax as b2j
    if getattr(b2j, "_legalize_installed", False):
        return
    orig = b2j.compile_bir_kernel

    def wrapped(bir_json, tmpdir, neff_name="file.neff"):
        return orig(legalize_bir_waits(bir_json), tmpdir, neff_name)

    b2j.compile_bir_kernel = wrapped
    b2j._legalize_installed = True


F32 = mybir.dt.float32
F16 = mybir.dt.float16
I32 = mybir.dt.int32
AF = mybir.ActivationFunctionType
ALU = mybir.AluOpType

B, L, D, H, DFF, MODES, ELAYERS = 64, 512, 512, 8, 2048, 32, 2
VOCAB, NUM_APP, KERNEL = 10000, 10000, 25
E = D // H  # 64
NCORES = 8
BL = B // NCORES  # 8 local batch
NT = BL * L       # 4096 local tokens
f16 = np.float16


# ---------------------------------------------------------------- host prep

def _movavg_matrix():
    """M[l_src, l_out]: weight of x[l_src] in moving_avg[l_out], including
    edge replication (pad (K-1)//2 each side with edge values)."""
    M = np.zeros((L, L), np.float64)
    pad = (KERNEL - 1) // 2
    for lo in range(L):
        for j in range(lo - pad, lo + pad + 1):
            M[min(max(j, 0), L - 1), lo] += 1.0 / KERNEL
    return M.astype(np.float32)


def _dft_c4():
    """C4 [128, (4 c, 64)]: stationary for DFT.  col j<32: cos(2pi*l*j/L);
    j>=32: -sin(2pi*l*(j-32)/L) with l = c*128+p."""
    out = np.zeros((128, 4, 64), np.float32)
    for c in range(4):
        lv = c * 128 + np.arange(128)
        for m in range(MODES):
            ang = 2.0 * np.pi * lv * m / L
            out[:, c, m] = np.cos(ang)
            out[:, c, 32 + m] = -np.sin(ang)
    return out.astype(f16)


def _idft_d():
    """Dstack [64, 512]: rows m<32: sc(m)*cos(2pi*m*l'/L); rows 32+m:
    -sc(m)*sin(...), sc = (2-delta_m0)/L."""
    out = np.zeros((64, L), np.float32)
    lp = np.arange(L)
    for m in range(MODES):
        sc = (1.0 if m == 0 else 2.0) / L
        ang = 2.0 * np.pi * m * lp / L
        out[m] = sc * np.cos(ang)
        out[32 + m] = -sc * np.sin(ang)
    return out.astype(f16)


def prep_weights(inp):
    """Pre-arrange all weights into SBUF-shaped host arrays (fp16)."""
    w = {}
    w["embt"] = (np.asarray(inp["app_emb_w"], np.float32)
                 + np.asarray(inp["time_b"], np.float32)[None, :]
                 ).astype(f16)
    w["tw_rep"] = np.broadcast_to(
        np.asarray(inp["time_w"], np.float32), (128, D)).astype(f16).copy()
    w["c4"] = _dft_c4()
    w["dstack"] = _idft_d()

    # A' = I - M folded decomp: res = A'^T-ish banded matmul of x.
    M = _movavg_matrix()
    A = np.eye(L, dtype=np.float32) - M
    adiag = np.zeros((128, 4, 128), np.float32)
    for c in range(4):
        adiag[:, c, :] = A[c * 128:(c + 1) * 128, c * 128:(c + 1) * 128]
    w["adiag"] = adiag.astype(f16)
    # band tiles zero-padded to 32/64-aligned partition bases (matmul
    # requires base_partition in {0, 32, 64} matching on both operands)
    aup = np.zeros((32, 3, 128), np.float32)    # tile (co+1, co): rows 0:12
    alo = np.zeros((128, 3, 128), np.float32)   # tile (co-1, co): rows 116:128
    for co in range(3):
        aup[0:12, co, :] = A[(co + 1) * 128:(co + 1) * 128 + 12,
                             co * 128:(co + 1) * 128]
    for co in range(1, 4):
        alo[116:128, co - 1, :] = A[co * 128 - 12:co * 128,
                                    co * 128:(co + 1) * 128]
    w["aup"] = aup.astype(f16)
    w["alo"] = alo.astype(f16)

    Wq = np.asarray(inp["Wq"], np.float32)
    # rank-1 time-term compensation for layer 0: the layer-0 bounce/transpose
    # carries the embedding only; q_time = xtime (x) (tw @ Wq0.T) is added in
    # the DFT psum via one rank-1 matmul per b (xstb (x) twq0).
    w["twq0"] = (np.asarray(inp["time_w"], np.float32)
                 @ Wq[0].T).reshape(1, D).astype(f16)
    Wo = np.asarray(inp["Wo"], np.float32)
    wr = np.asarray(inp["four_wr"], np.float32)
    wi = np.asarray(inp["four_wi"], np.float32)
    c1 = np.asarray(inp["conv1_w"], np.float32)
    c2 = np.asarray(inp["conv2_w"], np.float32)
    bq = np.asarray(inp["bq"], np.float32)

    for l in range(ELAYERS):
        wqt = np.zeros((128, 4, D), np.float32)
        for k in range(4):
            wqt[:, k, :] = Wq[l].T[k * 128:(k + 1) * 128, :]
        w[f"wqt{l}"] = wqt.astype(f16)
        # bq is folded into the DFT output bias: xs += sum_l C[l,m] * bq
        # (only the m=0 cos row survives: 512*bq).  bo cancels exactly in
        # series_decomp ((I-M) of a constant is 0), so the +bo add is dropped.
        xs_bias = np.zeros((64, 2, H, 32), np.float32)   # [m_ri,(q,h,i32)]
        xs_bias[0, :, :, :] = (512.0 * bq[l].reshape(H, 2, 32)
                               ).transpose(1, 0, 2)
        w[f"xsb{l}"] = xs_bias

        # m-chunk outermost so each per-mh DMA slice is contiguous
        wc = np.zeros((128, 8, H, 4, 128), np.float32)
        for h in range(H):
            for m in range(MODES):
                mh, mi = m // 4, m % 4
                wc[0:64, mh, h, mi, 0:64] = wr[l, h, :, :, m]     # i x o
                wc[0:64, mh, h, mi, 64:128] = wi[l, h, :, :, m]
                wc[64:128, mh, h, mi, 0:64] = -wi[l, h, :, :, m]
                wc[64:128, mh, h, mi, 64:128] = wr[l, h, :, :, m]
        w[f"wc{l}"] = wc.astype(f16)

        wot = np.zeros((128, 4, D), np.float32)
        for jc in range(4):
            wot[:, jc, :] = Wo[l].T[jc * 128:(jc + 1) * 128, :]
        w[f"wot{l}"] = wot.astype(f16)

        # f-half outermost so each per-fh DMA slice is contiguous
        c1s = np.zeros((128, 2, 4, 8, 128), np.float32)
        for dc in range(4):
            for ft in range(16):
                c1s[:, ft // 8, dc, ft % 8, :] = \
                    c1[l][ft * 128:(ft + 1) * 128,
                          dc * 128:(dc + 1) * 128].T
        w[f"c1{l}"] = c1s.astype(f16)

        c2t = np.zeros((128, 16, D), np.float32)
        for fc in range(16):
            c2t[:, fc, :] = c2[l].T[fc * 128:(fc + 1) * 128, :]
        w[f"c2{l}"] = c2t.astype(f16)

    w["normwT"] = np.asarray(
        inp["norm_w"], np.float32).reshape(4, 128).T.astype(f16).copy()
    pw = np.asarray(inp["proj_w"], np.float32)       # [10000, 536]
    pwt = np.zeros((128, 5, NUM_APP), np.float32)
    for ck in range(4):
        pwt[:, ck, :] = pw.T[ck * 128:(ck + 1) * 128, :]
    pwt[0:24, 4, :] = pw.T[512:536, :]               # time_vec columns
    pwt[24, 4, :] = np.asarray(inp["proj_b"], np.float32)  # bias row
    w["pwt"] = pwt.astype(f16)
    # lastw[p, c]: weights so that sum_l lastw[l] * xh0[l, d] =
    # xh0[last, d] - mean_l(xh0[., d])
    lastw = np.full((128, 4), -1.0 / L, np.float32)
    lastw[127, 3] += 1.0
    w["lastw"] = lastw.astype(f16)
    return w


def _dft_full():
    """C [512, 64]: C[l, m] = cos(2pi l m / L), C[l, 32+m] = -sin(...)."""
    out = np.zeros((L, 64), np.float32)
    lv = np.arange(L)
    for m in range(MODES):
        ang = 2.0 * np.pi * lv * m / L
        out[:, m] = np.cos(ang)
        out[:, 32 + m] = -np.sin(ang)
    return out


def make_in_maps(inputs, w):
    x_app = np.asarray(inputs["x_app"])
    x_time = np.asarray(inputs["x_time"], np.float32)
    time_vecs = np.asarray(inputs["time_vecs"], np.float32)
    C = _dft_full()
    in_maps = []
    for core in range(NCORES):
        sl = slice(core * BL, (core + 1) * BL)
        m = {"x_app": np.ascontiguousarray(x_app[sl]).astype(np.int32),
             "x_time": np.ascontiguousarray(x_time[sl]),
             "xstb": (x_time[sl] @ C).reshape(1, BL * 64).astype(f16),
             "tv_last": np.ascontiguousarray(time_vecs[sl, L - 1, :])}
        m.update(w)
        in_maps.append(m)
    return in_maps


# ---------------------------------------------------------------- builder

def build_nc(num_devices=NCORES, debug=False):
    nc = bass.Bass("TRN2", target_bir_lowering=False, debug=False,
                   num_devices=num_devices)
    P = {}

    def param(name, shape, dtype):
        P[name] = nc.declare_dram_parameter(name, list(shape), dtype,
                                            isOutput=False)

    param("x_app", [BL, L], I32)
    param("x_time", [BL, L], F32)
    param("xstb", [1, BL * 64], F16)
    param("twq0", [1, D], F16)
    param("tv_last", [BL, 24], F32)
    param("embt", [VOCAB, D], F16)
    param("tw_rep", [128, D], F16)
    param("c4", [128, 4, 64], F16)
    param("dstack", [64, L], F16)
    param("adiag", [128, 4, 128], F16)
    param("aup", [32, 3, 128], F16)
    param("alo", [128, 3, 128], F16)
    for l in range(ELAYERS):
        param(f"wqt{l}", [128, 4, D], F16)
        param(f"xsb{l}", [64, 2, H, 32], F32)
        param(f"wc{l}", [128, 8, H, 4, 128], F16)
        param(f"wot{l}", [128, 4, D], F16)
        param(f"c1{l}", [128, 2, 4, 8, 128], F16)
        param(f"c2{l}", [128, 16, D], F16)
    param("normwT", [128, 4], F16)
    param("pwt", [128, 5, NUM_APP], F16)
    param("lastw", [128, 4], F16)

    OUT = nc.declare_dram_parameter("out", [BL, NUM_APP], F16, isOutput=True)
    xrow = nc.dram_tensor("xrow", [NT, D], F16)   # transpose bounce

    dbg = {}

    def dbg_dump(name, ap, shape, dtype=F16):
        if debug:
            dbg[name] = nc.declare_dram_parameter(
                "dbg_" + name, list(shape), dtype, isOutput=True)
            nc.sync.dma_start(dbg[name].ap(), ap)

    with tile.TileContext(nc) as tc:
        with tc.tile_pool(name="spine", bufs=1) as p_spine, \
             tc.tile_pool(name="bigT", bufs=1) as p_bigT, \
             tc.tile_pool(name="qsb", bufs=2) as p_qsb, \
             tc.tile_pool(name="four", bufs=1) as p_four, \
             tc.tile_pool(name="wcb", bufs=2) as p_wcb, \
             tc.tile_pool(name="y1p", bufs=1) as p_y1, \
             tc.tile_pool(name="wts", bufs=1) as p_wts, \
             tc.tile_pool(name="wly", bufs=1) as p_wly, \
             tc.tile_pool(name="ffw", bufs=2) as p_ffw, \
             tc.tile_pool(name="sml", bufs=1) as p_sml, \
             tc.tile_pool(name="pws", bufs=2) as p_pws, \
             tc.tile_pool(name="ps", bufs=3, space="PSUM") as p_ps, \
             tc.tile_pool(name="psd", bufs=3, space="PSUM") as p_psd, \
             tc.tile_pool(name="ps2", bufs=2, space="PSUM") as p_ps2:

            # ---------------- early loads: b0's embedding chain first ----
            idx_sb = p_sml.tile([128, 32], I32, tag="idxsb")
            # idx_sb[p, b*4+c] = x_app[b, c*128+p]  (= token t's app id)
            nc.sync.dma_start(
                idx_sb[:],
                P["x_app"].ap().rearrange("b (c p) -> p (b c)", p=128))
            xt_t = p_sml.tile([128, 32], F32, tag="xt")
            nc.sync.dma_start(
                xt_t[:],
                P["x_time"].ap().rearrange("b (c p) -> p (b c)", p=128))
            tw_t = p_wts.tile([128, D], F16, tag="twrep")
            nc.sync.dma_start(tw_t[:], P["tw_rep"][:])
            xstb_t = p_sml.tile([1, BL * 64], F16, tag="xstb")
            nc.sync.dma_start(xstb_t[:], P["xstb"][:])
            twq0_t = p_sml.tile([1, D], F16, tag="twq0")
            nc.sync.dma_start(twq0_t[:], P["twq0"][:])
            c4_t = p_wts.tile([128, 4, 64], F16, tag="c4")
            nc.sync.dma_start(c4_t[:], P["c4"][:])

            def load_consts():
                # bulk constants, deferred until after layer-0 Phase A has
                # started so they don't delay the first Q-proj
                dst_t = p_wts.tile([64, L], F16, tag="dstack")
                nc.sync.dma_start(dst_t[:], P["dstack"][:])
                adiag_t = p_wts.tile([128, 4, 128], F16, tag="adiag")
                nc.sync.dma_start(adiag_t[:], P["adiag"][:])
                aup_t = p_wts.tile([32, 3, 128], F16, tag="aup")
                nc.sync.dma_start(aup_t[:], P["aup"][:])
                alo_t = p_wts.tile([128, 3, 128], F16, tag="alo")
                nc.sync.dma_start(alo_t[:], P["alo"][:])
                lastw_t = p_wts.tile([128, 4], F16, tag="lastw")
                nc.sync.dma_start(lastw_t[:], P["lastw"][:])
                normwT_t = p_wts.tile([128, 4], F16, tag="normwT")
                nc.sync.dma_start(normwT_t[:], P["normwT"][:])
                return dst_t, adiag_t, aup_t, alo_t, lastw_t, normwT_t

            x_S = p_spine.tile([128, BL, 4, D], F16, tag="spine")

            modes_sb = p_four.tile([128, 2, 2, 64, MODES], F16, tag="modes")
            # xsT b-columns 8..31 are zero-padded so the mode-mix matmuls
            # fill whole 32-row PSUM blocks (merged copies read no garbage)
            xsT = p_four.tile([128, 32, H, 32], F16, tag="xsT")
            nc.vector.memset(xsT[:], 0.0)

            sum_x = p_sml.tile([128, 32], F32, tag="sumx")
            sum_sq = p_sml.tile([128, 32], F32, tag="sumsq")
            rstd = p_sml.tile([128, 32], F32, tag="rstd")
            lastT = p_sml.tile([128, 4, 8], F16, tag="lastT")
            lastT45 = p_sml.tile([32, 8], F16, tag="lastT45")


            def bounce_b(b, eng=None):
                """x_S[:, b] -> DRAM rows (for the xbar transposes)."""
                (eng or nc.sync).dma_start(
                    xrow.ap()[b * 512:(b + 1) * 512, :]
                    .rearrange("(c p) d -> p c d", p=128),
                    x_S[:, b, :, :])

            def transpose_b(dst, b):
                for k in range(4):
                    nc.sync.dma_start_transpose(
                        dst[:, k, b * 512:(b + 1) * 512],
                        xrow.ap()[b * 512:(b + 1) * 512,
                                  k * 128:(k + 1) * 128])

            def decomp_b(nc, b):
                """x_S[:, b] <- (I-M) applied over l to x_S[:, b].
                All matmuls read the original x_S[:, b] (program order), the
                copies write the banded-transform result back."""
                pss = []
                for co in range(4):
                    src = x_S[:, b, :, :]
                    ps_a = p_psd.tile([128, D], F32, tag="psd")
                    mms = [(adiag_t[:, co, :], src[:, co, :])]
                    if co < 3:
                        mms.append((aup_t[:, co, :], src[0:32, co + 1, :]))
                    if co > 0:
                        mms.append((alo_t[64:128, co - 1, :],
                                    src[64:128, co - 1, :]))
                    for i, (lhsT, rhs) in enumerate(mms):
                        nc.tensor.matmul(ps_a[:], lhsT, rhs,
                                         start=(i == 0),
                                         stop=(i == len(mms) - 1))
                    pss.append(ps_a)
                for co in range(4):
                    nc.vector.tensor_copy(x_S[:, b, co, :], pss[co][:])

            # ---------------- layers ----------------
            for l in range(ELAYERS):
                wqt_t = p_wly.tile([128, 4, D], F16, tag="wqt")
                nc.sync.dma_start(wqt_t[:], P[f"wqt{l}"][:])
                xsb_t = p_wly.tile([64, 2, H, 32], F32, tag="xsb")
                nc.sync.dma_start(xsb_t[:], P[f"xsb{l}"][:])
                wot_t = p_wly.tile([128, 4, D], F16, tag="wot")
                nc.sync.dma_start(wot_t[:], P[f"wot{l}"][:])

                # ---- per-b: (emb) -> bounce -> transpose -> Q-proj ->
                #      DFT -> shuffle1 ----
                x_T = p_bigT.tile([128, 4, NT], F16, tag="bigT")
                for b in range(BL):
                    if l == 0:
                        for c in range(4):
                            j = b * 4 + c
                            nc.gpsimd.indirect_dma_start(
                                out=x_S[:, b, c, :], out_offset=None,
                                in_=P["embt"][:],
                                in_offset=bass.IndirectOffsetOnAxis(
                                    ap=idx_sb[:, j:j + 1], axis=0))
                    # layer 0: bounce the embedding only -- the time term is
                    # compensated in the DFT (rank-1) and added to the spine
                    # off the critical path below
                    bounce_b(b)
                    transpose_b(x_T, b)

                    q_sb = p_qsb.tile([128, 4, D], F16, tag="qsb")
                    for c in range(4):
                        t0 = (b * 4 + c) * 128
                        ps_q = p_ps.tile([128, D], F32, tag="ps")
                        for k in range(4):
                            nc.tensor.matmul(
                                ps_q[:], x_T[:, k, t0:t0 + 128],
                                wqt_t[:, k, :],
                                start=(k == 0), stop=(k == 3))
                        nc.scalar.copy(q_sb[:, c, :], ps_q[:])
                    ps_xs = p_ps2.tile([64, D], F32, tag="psxs")
                    for c in range(4):
                        nc.tensor.matmul(ps_xs[:], c4_t[:, c, :],
                                         q_sb[:, c, :],
                                         start=(c == 0),
                                         stop=(c == 3 and l > 0))
                    if l == 0:
                        nc.tensor.matmul(
                            ps_xs[:], xstb_t[0:1, b * 64:(b + 1) * 64],
                            twq0_t[:], start=False, stop=True)
                    # psum free order (h, q, i32) -> xs_b[:, q, h, i32]
                    xs_b = p_qsb.tile([64, 2, H, 32], F16, tag="xsb", bufs=1)
                    nc.vector.tensor_add(
                        xs_b[:].rearrange("p q h i -> p h q i"),
                        ps_xs[:].rearrange("p (h q i) -> p h q i", h=8, q=2),
                        xsb_t[:].rearrange("p q h i -> p h q i"))

                    # shuffle1: xs -> xsT [(ri,i), (b, h, m)]
                    for ri in range(2):
                        for q in range(2):
                            r0 = ri * 64 + q * 32
                            nc.vector.transpose(
                                xsT[r0:r0 + 32, b, :, :]
                                .rearrange("p h m -> p (h m)"),
                                xs_b[ri * 32:ri * 32 + 32, q, :, :]
                                .rearrange("p h i -> p (h i)"))
                    if l == 0:
                        # spine time term (reads the bounced x_S after the
                        # bounce DMA; needed before the Wo-add)
                        for c in range(4):
                            j = b * 4 + c
                            nc.vector.scalar_tensor_tensor(
                                out=x_S[:, b, c, :], in0=tw_t[:],
                                scalar=xt_t[:, j:j + 1], in1=x_S[:, b, c, :],
                                op0=ALU.mult, op1=ALU.add)

                if l == 0:
                    (dst_t, adiag_t, aup_t, alo_t, lastw_t,
                     normwT_t) = load_consts()
                dbg_dump(f"xa{l}", x_S[:], [128, BL, 4, D])

                # ---- mode mix ----
                # psum tile (mh, hp): [128, 512]; rows 32g+b (g = h//2);
                # cols mi*128 + (ri*64 + o).  modes_sb rows sparse 32g+b.
                for mh in range(8):
                    m0 = mh * 4
                    wc_t = p_wcb.tile([128, H, 4, 128], F16, tag="wcb")
                    (nc.sync if l == 0 else nc.scalar).dma_start(
                        wc_t[:], P[f"wc{l}"][:, mh])
                    for hp in range(2):
                        ps_mm = p_ps.tile([128, D], F32, tag="ps")
                        for g in range(4):
                            h = g * 2 + hp
                            for mi in range(4):
                                nc.tensor.matmul(
                                    ps_mm[32 * g:32 * g + 32,
                                          mi * 128:(mi + 1) * 128],
                                    xsT[:, :, h, m0 + mi],
                                    wc_t[:, h, mi, :],
                                    start=True, stop=True,
                                    tile_position=(0, 32 * g))
                        # single merged copy for all 4 g-blocks (rows 32g+bb;
                        # rows 8..32 of each block stay zero from the memset)
                        nc.scalar.copy(
                            modes_sb[:, hp, :, :, m0:m0 + 4]
                            .rearrange("p r o m -> p m r o"),
                            ps_mm[:].rearrange("p (m r o) -> p m r o",
                                               m=4, r=2))

                # ---- shuffle2 / iDFT / Wo, pipelined in g-pair halves:
                # the DVE stream-transposes of half gp overlap the PE
                # iDFT+Wo matmuls of half gp-1 ----
                modes_T = p_four.tile([64, 2, 64, 32], F16, tag="modesT")
                mscr = p_four.tile([64, 2, 64, 32], F16, tag="mscr")
                y_S = p_bigT.tile([128, 4, BL, D], F16, tag="bigT")
                mt5 = modes_T[:].rearrange("p hp o (g bb) -> p g hp o bb",
                                           g=4)
                for gp in range(2):
                    for g in (2 * gp, 2 * gp + 1):
                        for ri in range(2):
                            for hp in range(2):
                                nc.vector.transpose(
                                    mscr[ri * 32:(ri + 1) * 32, hp, :, :]
                                    .rearrange("p o w -> p (o w)"),
                                    modes_sb[32 * g:32 * (g + 1), hp, ri,
                                             :, :]
                                    .rearrange("p o m -> p (o m)"))
                        nc.vector.tensor_copy(
                            modes_T[:, :, :, g * 8:(g + 1) * 8],
                            mscr[:, :, :, 0:8])
                    # iDFT for output cols h in {4gp..4gp+3}, two b at a time
                    for lc in range(4):
                        for bp in range(4):
                            ps_y = p_ps.tile([128, 512], F32, tag="ps")
                            nc.tensor.matmul(
                                ps_y[:],
                                dst_t[:, lc * 128:(lc + 1) * 128],
                                mt5[:, 2 * gp:2 * gp + 2, :, :,
                                    2 * bp:2 * bp + 2],
                                start=True, stop=True)
                            # psum cols (g2, hp, o, b2) -> y_S d-cols per b
                            nc.scalar.copy(
                                y_S[:, lc, 2 * bp:2 * bp + 2,
                                    gp * 256:(gp + 1) * 256]
                                .rearrange("p b d -> p d b"),
                                ps_y[:].rearrange("p (d b) -> p d b", b=2))
                    # Wo for token tiles ht in {2gp, 2gp+1}
                    # (token l = (h,e); ht tile = h-pair = g)
                    for b in range(BL):
                        for ht in (2 * gp, 2 * gp + 1):
                            ps_wo = p_ps.tile([128, D], F32, tag="ps")
                            for jc in range(4):
                                nc.tensor.matmul(
                                    ps_wo[:],
                                    y_S[:, jc, b, ht * 128:(ht + 1) * 128],
                                    wot_t[:, jc, :],
                                    start=(jc == 0), stop=(jc == 3))
                            nc.vector.tensor_add(x_S[:, b, ht, :],
                                                 x_S[:, b, ht, :], ps_wo[:])

                # ---- back half, per-b pipeline: decomp1 -> bounce ->
                # transpose -> FFN (both f-halves) -> decomp2 (-> LN) ----
                r1T = p_bigT.tile([128, 4, NT], F16, tag="bigT")
                c1h, c2h = [], []
                for fh in range(2):
                    qeng = nc.sync if l == 0 else nc.scalar
                    c1_t = p_ffw.tile([128, 4, 8, 128], F16, tag="c1")
                    qeng.dma_start(c1_t[:], P[f"c1{l}"][:, fh])
                    c2_t = p_ffw.tile([128, 8, D], F16, tag="c2")
                    qeng.dma_start(
                        c2_t[:], P[f"c2{l}"][:, fh * 8:(fh + 1) * 8, :])
                    c1h.append(c1_t)
                    c2h.append(c2_t)
                for b in range(BL):
                    decomp_b(nc, b)
                    bounce_b(b)
                    transpose_b(r1T, b)
                dbg_dump(f"xd{l}", x_S[:], [128, BL, 4, D])

                def emit_row(b):
                    # column b of lastT: xh[last] - mean_l(xh), built in
                    # T-orientation (d on partitions); emitted one b late so
                    # the PE never waits on the LN DVE chain
                    ps_m = p_ps.tile([128, 4], F32, tag="ps")
                    for ck in range(4):
                        for c in range(4):
                            nc.tensor.matmul(
                                ps_m[:, ck:ck + 1],
                                x_S[:, b, c, ck * 128:(ck + 1) * 128],
                                lastw_t[:, c:c + 1],
                                start=(c == 0), stop=(c == 3))
                    nc.vector.tensor_tensor(
                        out=lastT[:, :, b], in0=ps_m[:], in1=normwT_t[:],
                        op=ALU.mult)

                for b in range(BL):
                    for fh in range(2):
                        y1_t = p_y1.tile([128, 8, D], F16, tag="y1")
                        for ft in range(8):
                            ps_f1 = p_ps.tile([128, D], F32, tag="ps")
                            for dc in range(4):
                                nc.tensor.matmul(
                                    ps_f1[:], c1h[fh][:, dc, ft, :],
                                    r1T[:, dc, b * 512:(b + 1) * 512],
                                    start=(dc == 0), stop=(dc == 3))
                            nc.scalar.activation(y1_t[:, ft, :], ps_f1[:],
                                                 AF.Relu)
                        for ht in range(4):
                            ps_f2 = p_ps.tile([128, D], F32, tag="ps")
                            for fc in range(8):
                                nc.tensor.matmul(
                                    ps_f2[:],
                                    y1_t[:, fc, ht * 128:(ht + 1) * 128],
                                    c2h[fh][:, fc, :],
                                    start=(fc == 0), stop=(fc == 7))
                            nc.vector.tensor_add(x_S[:, b, ht, :],
                                                 x_S[:, b, ht, :], ps_f2[:])
                    decomp_b(nc, b)
                    if l == ELAYERS - 1:
                        # LN stats + in-place normalize + final row, per b
                        j0 = 4 * b
                        sx = slice(j0, j0 + 4)
                        nc.vector.tensor_reduce(
                            sum_x[:, sx],
                            x_S[:, b, :, :], mybir.AxisListType.X, ALU.add)
                        for c in range(4):
                            # y1_t (fh=1) is dead; reuse it as the dump
                            nc.scalar.activation(
                                y1_t[:, 0, :], x_S[:, b, c, :], AF.Square,
                                accum_out=sum_sq[:, j0 + c:j0 + c + 1])
                        mu = sum_x  # in place: sum_x -> mean
                        nc.vector.tensor_scalar_mul(mu[:, sx], sum_x[:, sx],
                                                    1.0 / D)
                        nc.vector.tensor_mul(rstd[:, sx], mu[:, sx],
                                             mu[:, sx])
                        nc.vector.scalar_tensor_tensor(
                            out=rstd[:, sx], in0=sum_sq[:, sx],
                            scalar=1.0 / D, in1=rstd[:, sx],
                            op0=ALU.mult, op1=ALU.subtract)
                        nc.vector.tensor_scalar_add(rstd[:, sx], rstd[:, sx],
                                                    1e-5)
                        nc.scalar.activation(rstd[:, sx], rstd[:, sx],
                                             AF.Sqrt)
                        nc.vector.reciprocal(rstd[:, sx], rstd[:, sx])
                        for c in range(4):
                            # xh0 in place on the (now dead) spine
                            nc.vector.tensor_scalar(
                                x_S[:, b, c, :], x_S[:, b, c, :],
                                mu[:, j0 + c:j0 + c + 1],
                                rstd[:, j0 + c:j0 + c + 1],
                                ALU.subtract, ALU.mult)


            dbg_dump("xfin", x_S[:], [128, BL, 4, D])

            # ---------------- final projection ----------------
            for b in range(BL):
                emit_row(b)
            # lastT45 rows 0:24 = time_vec columns, row 24 = const 1 (bias
            # row of the fused pwt); rows 25:32 x zero weight rows
            nc.vector.memset(lastT45[:], 1.0)
            nc.gpsimd.dma_start(lastT45[0:24, :],
                                P["tv_last"].ap().rearrange("b t -> t b"))
            dbg_dump("lastT", lastT[:], [128, 4, 8])
            dbg_dump("lastT45", lastT45[:], [32, 8])
            dbg_dump("rstd", rstd[:], [128, 32], F32)
            dbg_dump("sumsq", sum_sq[:], [128, 32], F32)

            # projection over 256-wide vocab slices; bias is folded into the
            # matmul (bias row of pwt x const-1 col of lastT45); OUT written
            # in 512-wide pairs
            for vs in range(20):
                v0 = vs * 512
                vw = min(512, NUM_APP - v0)
                pw_t = p_ffw.tile([128, 5, 512], F16, tag="pwbig")
                nc.sync.dma_start(pw_t[:, :, 0:vw],
                                  P["pwt"][:, :, v0:v0 + vw])
                ps_p = p_ps2.tile([8, 512], F32, tag="psxs")
                for ck in range(4):
                    nc.tensor.matmul(ps_p[:, 0:vw], lastT[:, ck, :],
                                     pw_t[:, ck, 0:vw],
                                     start=(ck == 0), stop=False)
                nc.tensor.matmul(ps_p[:, 0:vw], lastT45[:],
                                 pw_t[0:32, 4, 0:vw],
                                 start=False, stop=True)
                sc_t = p_pws.tile([8, 512], F16, tag="sc", bufs=1)
                nc.vector.tensor_copy(sc_t[:, 0:vw], ps_p[:, 0:vw])
                nc.sync.dma_start(OUT[:, v0:v0 + vw], sc_t[:, 0:vw])

    return nc


# ---------------------------------------------------------------- runner

_CACHED = {}


def kernel(**inputs):
    install()

    w = prep_weights(inputs)
    if "nc" not in _CACHED:
        _CACHED["nc"] = build_nc()
    nc = _CACHED["nc"]
    in_maps = make_in_maps(inputs, w)

    res = run_bass_kernel_spmd(nc, in_maps, list(range(NCORES)))
    out = np.concatenate([res.results[i]["out"] for i in range(NCORES)],
                         axis=0)
    return out.astype(np.float32)


if __name__ == "__main__":
    import reference
    inp = {k: np.asarray(v) for k, v in reference.setup_inputs().items()}
    got = kernel(**inp)
    exp = np.asarray(reference.reference(**reference.setup_inputs()))
    err = np.linalg.norm(got - exp) / np.linalg.norm(exp)
    print("Relative error:", err)
